# revision 4
# baseline (speedup 1.0000x reference)
"""DGCNN (dynamic edge conv x2 + classifier) Trainium2 Bass kernel.

Sharding: data-parallel over the 8 point clouds -> 8 NeuronCores.
Each core runs the full per-cloud pipeline:
  conv1: kNN in 3-D, edge MLP 6->64->64->64, max over K=20
  conv2: kNN in 64-D feature space, edge MLP 128->128->128->256, max over K
  head : 256->512, global max pool, 512->256->256->40, log_softmax

Key per-core implementation ideas:
  * kNN scores via one augmented matmul: [x,1] @ [-2x; |x|^2]^T.
  * composite sort keys: each u32 word = fp16(-d2) in the high 16 bits
    (written by the scalar engine straight from PSUM with a strided AP)
    and a persistent u16 column-index iota in the low 16 bits.  Read as
    f32, lexicographic float order ranks by distance with deterministic
    index tie-breaks, so DVE max8 alone yields both value and index --
    no max_index scans, and neighbor indices pop out with a bitwise AND.
  * top-24 via 3 rounds of max8; match_replace writes into a scratch
    copy so the key tile (and its iota) is never clobbered.
  * per-round batched indirect-DMA gathers (8 offsets per call) instead
    of one call per neighbor: 3 GPSIMD descriptor-gen ops per tile.
  * edge first layer decomposed: z1 = relu(a_i + v_j) with
    a = x@(W1a-W1b)+b1 (point-major, one small matmul per tile) and
    v = x@W1b gathered from DRAM; the broadcast add runs point-major as
    a single DVE op over all 20 neighbors before the PE transposes.
  * layer-3 outputs for a whole tile land in one multi-bank PSUM tile;
    the max over K collapses to one strided tensor_reduce per half.
"""

import os
import sys
import numpy as np

for _p in ("/opt/trn_rl_repo",):
    if _p not in sys.path:
        sys.path.insert(0, _p)

N = 2048          # points per cloud
NCLOUD = 8
P = 128           # partition tile
NT = N // P       # 16 row tiles
KNN = 20
KSEL = 24         # 3 rounds x 8
NEG_BIG = -3.0e38
NUM_CLASSES = 40

# matmul dtype knobs: None -> plain float32; "f32r" -> float32r fast path
MM_FAST_MLP = True     # edge-MLP layers 2/3 + classifier matmuls
MM_FAST_SCORES = False  # kNN score matmuls (selection-critical, keep exact)

_PROGRAM_CACHE = {}


def _build_program():
    import concourse.bass as bass
    import concourse.bacc as bacc
    import concourse.tile as tile
    from concourse import mybir

    f32 = mybir.dt.float32
    f32r = mybir.dt.float32r
    f16 = mybir.dt.float16
    bf16 = mybir.dt.bfloat16
    u16 = mybir.dt.uint16
    u32 = mybir.dt.uint32
    AX = mybir.AxisListType
    OP = mybir.AluOpType
    ACT = mybir.ActivationFunctionType

    def mm(ap):
        return ap.bitcast(f32r) if MM_FAST_MLP else ap

    def mms(ap):
        return ap.bitcast(f32r) if MM_FAST_SCORES else ap

    mmo = mm  # producer outputs feeding fast matmuls must round to f32r

    nc = bacc.Bacc("TRN2", target_bir_lowering=False, debug=False)

    # ---------------- I/O ----------------
    def din(name, shape):
        return nc.dram_tensor(name, list(shape), f32, kind="ExternalInput").ap()

    pos = din("pos", [N, 3])
    c1w1 = din("c1w1", [6, 64]);   c1b1 = din("c1b1", [64])
    c1w2 = din("c1w2", [64, 64]);  c1b2 = din("c1b2", [64])
    c1w3 = din("c1w3", [64, 64]);  c1b3 = din("c1b3", [64])
    c2w1 = din("c2w1", [128, 128]); c2b1 = din("c2b1", [128])
    c2w2 = din("c2w2", [128, 128]); c2b2 = din("c2b2", [128])
    c2w3 = din("c2w3", [128, 256]); c2b3 = din("c2b3", [256])
    l0w = din("l0w", [256, 512]);  l0b = din("l0b", [512])
    l1w = din("l1w", [512, 256]);  l1b = din("l1b", [256])
    l2w = din("l2w", [256, 256]);  l2b = din("l2b", [256])
    l3w = din("l3w", [256, NUM_CLASSES]); l3b = din("l3b", [NUM_CLASSES])
    ident = din("ident", [128, 128])

    out = nc.dram_tensor("out", [1, NUM_CLASSES], f32, kind="ExternalOutput").ap()

    with tile.TileContext(nc) as tc:
        from contextlib import ExitStack

        ctx = ExitStack()
        g = ctx.enter_context(tc.tile_pool(name="g", bufs=1))          # persistent
        dpool = ctx.enter_context(tc.tile_pool(name="dram", bufs=1, space="DRAM"))

        # persistent SBUF state
        ident_sb = g.tile([128, 128], f32)
        nc.sync.dma_start(ident_sb[:], ident[:, :])
        ident_bf = g.tile([128, 128], bf16)
        nc.scalar.copy(ident_bf[:, :], ident_sb[:, :])

        A1 = g.tile([4, N], f32)       # [x^T ; 1]
        B1 = g.tile([4, N], f32)       # [-2 x^T ; s]
        A2 = g.tile([65, N], f32)      # [x1^T ; 1]
        B2 = g.tile([65, N], f32)      # [-2 x1^T ; s2]
        x2Ta = g.tile([128, N], f32)   # conv2 out ch 0:128
        x2Tb = g.tile([128, N], f32)   # conv2 out ch 128:256
        nscol1 = g.tile([128, NT], f32)  # -s_i per tile column
        nscol2 = g.tile([128, NT], f32)
        vscratch = g.tile([128, N], f32, name="vscratch")  # v1T/x1sq/v2T staging
        apm1 = g.tile([128, NT * 64], f32, name="apm1")    # a_i point-major
        apm2 = g.tile([128, NT * 128], bf16, name="apm2")

        # composite-key ring: u32 word = fp16(-d2) << 16 | column index.
        # The u16 iota in the low halves persists across tiles and convs;
        # only the fp16 halves are rewritten (strided ACT store from PSUM).
        KB = 3
        keybufs = [g.tile([128, 2 * N], u16, name=f"key{r}") for r in range(KB)]
        for kb in keybufs:
            ev = kb.rearrange("p (n two) -> p n two", two=2)[:, :, 0:1]
            nc.gpsimd.iota(ev, pattern=[[1, N]], base=0, channel_multiplier=0)

        v1d = dpool.tile([N, 64], f32, name="v1d")
        v2d = dpool.tile([N, 128], bf16, name="v2d")

        # weights / biases.  Weights consumed by f32r matmuls are loaded into
        # a scratch tile and rounded into an f32r-typed tile with an ACT copy
        # (the BIR verifier requires every writer of an f32r matmul operand
        # to emit rounded data, so the DMA cannot write them directly).
        wraw = ctx.enter_context(tc.tile_pool(name="wraw", bufs=2))

        def load_w(name, shape, pieces, wdt=None):
            wdt = wdt or (f32r if MM_FAST_MLP else f32)
            if wdt != f32:
                raw = wraw.tile(list(shape), f32, name=name + "_raw", tag="wraw")
                for sl, srcap in pieces:
                    nc.sync.dma_start(raw[sl], srcap)
                t = g.tile(list(shape), wdt, name=name)
                nc.scalar.copy(t[:, :], raw[:, :])
            else:
                t = g.tile(list(shape), f32, name=name)
                for sl, srcap in pieces:
                    nc.sync.dma_start(t[sl], srcap)
            return t

        SALL = (slice(None), slice(None))
        w_c1w1a = g.tile([3, 64], f32); nc.sync.dma_start(w_c1w1a[:], c1w1[0:3, :])
        w_c1w1b = g.tile([3, 64], f32); nc.sync.dma_start(w_c1w1b[:], c1w1[3:6, :])
        w_c2w1a = g.tile([64, 128], f32); nc.sync.dma_start(w_c2w1a[:], c2w1[0:64, :])
        w_c2w1b = g.tile([64, 128], f32); nc.sync.dma_start(w_c2w1b[:], c2w1[64:128, :])
        w_c1w2 = load_w("w_c1w2", [64, 64], [(SALL, c1w2[:, :])])
        w_c1w3 = load_w("w_c1w3", [64, 64], [(SALL, c1w3[:, :])])
        w_c2w2 = load_w("w_c2w2", [128, 128], [(SALL, c2w2[:, :])], wdt=bf16)
        w_c2w3 = load_w("w_c2w3", [128, 256], [(SALL, c2w3[:, :])], wdt=bf16)
        w_l0w = load_w("w_l0w", [128, 1024],
                       [((slice(None), slice(0, 512)), l0w[0:128, :]),
                        ((slice(None), slice(512, 1024)), l0w[128:256, :])])
        w_l1w = load_w("w_l1w", [128, 1024],
                       [((slice(None), slice(c * 256, (c + 1) * 256)),
                         l1w[c * 128:(c + 1) * 128, :]) for c in range(4)])
        w_l2w = load_w("w_l2w", [128, 512],
                       [((slice(None), slice(0, 256)), l2w[0:128, :]),
                        ((slice(None), slice(256, 512)), l2w[128:256, :])])
        w_l3w = load_w("w_l3w", [128, 2 * NUM_CLASSES],
                       [((slice(None), slice(0, NUM_CLASSES)), l3w[0:128, :]),
                        ((slice(None), slice(NUM_CLASSES, 2 * NUM_CLASSES)), l3w[128:256, :])])

        # first-layer combined weights: a = x @ (W1a - W1b) + b  (point-major)
        W1comb = g.tile([4, 64], f32, name="W1comb")
        nc.vector.tensor_sub(W1comb[0:3, :], w_c1w1a[:, :], w_c1w1b[:, :])
        nc.sync.dma_start(W1comb[3:4, :], c1b1.rearrange("(o c) -> o c", o=1))
        W2comb = g.tile([65, 128], f32, name="W2comb")
        nc.vector.tensor_sub(W2comb[0:64, :], w_c2w1a[:, :], w_c2w1b[:, :])
        nc.sync.dma_start(W2comb[64:65, :], c2b1.rearrange("(o c) -> o c", o=1))

        def col(name, src, n):
            t = g.tile([n, 1], f32, name=name)
            nc.sync.dma_start(t[:, :], src.rearrange("(c o) -> c o", o=1))
            return t

        b_c1b2 = col("b_c1b2", c1b2, 64)
        b_c1b3 = col("b_c1b3", c1b3, 64)
        b_c2b2 = col("b_c2b2", c2b2, 128)
        b_c2b3 = g.tile([128, 2], f32)
        nc.sync.dma_start(b_c2b3[:, 0:1], c2b3.rearrange("(h c o) -> h c o", h=2, o=1)[0])
        nc.sync.dma_start(b_c2b3[:, 1:2], c2b3.rearrange("(h c o) -> h c o", h=2, o=1)[1])
        b_l0b = g.tile([128, 4], f32)
        for t_ in range(4):
            nc.sync.dma_start(b_l0b[:, t_:t_ + 1],
                              l0b.rearrange("(h c o) -> h c o", h=4, o=1)[t_])
        b_l1b = g.tile([128, 2], f32)
        for t_ in range(2):
            nc.sync.dma_start(b_l1b[:, t_:t_ + 1],
                              l1b.rearrange("(h c o) -> h c o", h=2, o=1)[t_])
        b_l2b = g.tile([128, 2], f32)
        for t_ in range(2):
            nc.sync.dma_start(b_l2b[:, t_:t_ + 1],
                              l2b.rearrange("(h c o) -> h c o", h=2, o=1)[t_])
        b_l3b = col("b_l3b", l3b, NUM_CLASSES)

        # engines cannot address partition bases 3/64 directly: stage a ones
        # row at partition 0 and DMA it into place
        ones_row = g.tile([1, N], f32, name="ones_row")
        nc.vector.memset(ones_row[:, :], 1.0)
        nc.sync.dma_start(A1[3:4, :], ones_row[:, :])
        nc.sync.dma_start(A2[64:65, :], ones_row[:, :])

        # =============== conv1 prep ===============
        with tc.tile_pool(name="prep", bufs=2) as pp, \
             tc.tile_pool(name="prep_ps", bufs=2, space="PSUM") as ppp:
            scol = g.tile([128, NT], f32, name="scol1_pos")
            for i in range(NT):
                isl = slice(i * P, (i + 1) * P)
                pt = pp.tile([128, 3], f32, name="pt")
                nc.sync.dma_start(pt[:], pos[isl, :])
                sq = pp.tile([128, 3], f32, name="sq")
                nc.scalar.activation(sq[:], pt[:], ACT.Square,
                                     accum_out=scol[:, i:i + 1])
                tp = ppp.tile([3, 128], f32, name="tp", space="PSUM", bufs=1)
                nc.tensor.transpose(tp[:], pt[:], ident_sb[:])
                nc.scalar.copy(A1[0:3, isl], tp[:])
            nc.scalar.mul(nscol1[:, :], scol[:, :], -1.0)
            nc.scalar.mul(B1[0:3, :], A1[0:3, :], -2.0)
            # s row: transpose scol [128, NT] -> [NT, 128], stage in SBUF, then
            # one cross-partition DMA into B1 row 3 (engines cannot shift
            # partitions; DMA can).
            stp = ppp.tile([NT, 128], f32, name="stp", space="PSUM", bufs=1)
            nc.tensor.transpose(stp[:], scol[:, :], ident_sb[:])
            srow_sb = pp.tile([NT, 128], f32, name="srow_sb")
            nc.scalar.copy(srow_sb[:, :], stp[:, :])
            nc.sync.dma_start(
                B1[3:4, :].rearrange("o (p n) -> o p n", p=NT), srow_sb[:, :])

            # a_i point-major: one small matmul per tile (A1 rows carry [x;1])
            for i in range(NT):
                isl = slice(i * P, (i + 1) * P)
                pa = ppp.tile([128, 64], f32, name="pa", space="PSUM")
                nc.tensor.matmul(pa[:], A1[0:4, isl], W1comb[:, :])
                nc.scalar.copy(apm1[:, i * 64:(i + 1) * 64], pa[:])

            # v1 = x @ W1b, channel-major; rows -> DRAM [N, 64]
            for c in range(4):
                cs = slice(c * 512, (c + 1) * 512)
                pv = ppp.tile([64, 512], f32, name="pv", space="PSUM")
                nc.tensor.matmul(pv[:], w_c1w1b[:, :], A1[0:3, cs])
                nc.scalar.copy(vscratch[0:64, cs], pv[:])
            for grp in range(4):
                vstage = pp.tile([128, 256], f32, name="vstage")
                for m in range(4):
                    i = grp * 4 + m
                    tvp = ppp.tile([128, 64], f32, name="tvp", space="PSUM")
                    nc.tensor.transpose(tvp[:], vscratch[0:64, i * P:(i + 1) * P],
                                        ident_sb[0:64, 0:64])
                    nc.vector.tensor_copy(vstage[:, m * 64:(m + 1) * 64], tvp[:])
                nc.sync.dma_start(
                    v1d[:, :].rearrange("(g m r) ch -> g r m ch", g=4, m=4)[grp],
                    vstage[:, :])

        # =============== edge-conv block (shared structure) ===============
        def edge_conv(conv, sp, spp):
            """conv=1: H=64 channels; conv=2: H=128 (256 out)."""
            if conv == 1:
                H, CON, WD, ED, idt = 64, 4, 64, f32, ident_sb
                Asb, Bsb, vd, nscol, apm = A1, B1, v1d, nscol1, apm1
                wl2, wl3 = w_c1w2, w_c1w3
                bl2, bl3 = b_c1b2, b_c1b3
                W = mm        # f32r bitcast for conv1 MLP
            else:
                H, CON, WD, ED, idt = 128, 65, 128, bf16, ident_bf
                Asb, Bsb, vd, nscol, apm = A2, B2, v2d, nscol2, apm2
                wl2, wl3 = w_c2w2, w_c2w3
                bl2, bl3 = b_c2b2, b_c2b3
                W = lambda ap: ap   # tiles already bf16
            nhalf = 1 if conv == 1 else 2

            state = {}

            def stage_scores(i):
                isl = slice(i * P, (i + 1) * P)
                key = keybufs[i % KB]
                keyh = key.bitcast(f16).rearrange("p (n two) -> p n two", two=2)
                for c in range(4):
                    cs = slice(c * 512, (c + 1) * 512)
                    psc = spp.tile([128, 512], f32, name="psc", tag="psc", bufs=1)
                    nc.tensor.matmul(psc[:, :],
                                     mms(Asb[0:CON, isl]), mms(Bsb[0:CON, cs]))
                    # fp16(-d2) into the high u16 halves (strided store)
                    nc.scalar.activation(
                        keyh[:, cs, 1:2],
                        psc[:, :].rearrange("p (n o) -> p n o", o=1),
                        ACT.Identity, bias=nscol[:, i:i + 1], scale=-1.0)
                state[i] = {"key": key}

            def stage_topk(i):
                # 3 rounds of max8 over the composite keys; match_replace
                # writes into a scratch copy so the key iota survives.  The
                # neighbor index is the low 16 bits of each winning key; one
                # batched indirect gather per round (8 offsets per call).
                key32 = state[i]["key"].bitcast(f32)
                vals = sp.tile([128, KSEL], u32, name="vals", tag="vals", bufs=4)
                valsf = vals.bitcast(f32)
                idx = sp.tile([128, KSEL], u32, name="idx", tag="idx", bufs=4)
                scr = sp.tile([128, N], f32, name="scr", tag="scr", bufs=2)
                gath = sp.tile([128, KNN * WD], ED, name="gath", tag="gath", bufs=3)
                for r in range(3):
                    rs = slice(r * 8, (r + 1) * 8)
                    src = key32 if r == 0 else scr[:, :]
                    nc.vector.max(valsf[:, rs], src)
                    if r < 2:
                        nc.vector.match_replace(scr[:, :], valsf[:, rs], src,
                                                NEG_BIG)
                    nc.vector.tensor_scalar(idx[:, rs], vals[:, rs],
                                            0x7FF, None, op0=OP.bitwise_and)
                    # HW indirect DMA consumes ONE offset per destination
                    # partition (verified: extra offsets are ignored and the
                    # dest free size becomes a contiguous run), so issue one
                    # gather per neighbor slot.
                    for k in range(r * 8, min((r + 1) * 8, KNN)):
                        nc.gpsimd.indirect_dma_start(
                            out=gath[:, k * WD:(k + 1) * WD], out_offset=None,
                            in_=vd[:, :],
                            in_offset=bass.IndirectOffsetOnAxis(
                                ap=idx[:, k:k + 1], axis=0),
                        )
                state[i].update(gath=gath)

            def stage_mlp_a(i):
                # z1 = relu(a_i + v_j) point-major, then PE transposes to
                # channel-major; L2; L3 matmuls for half 0.
                isl = slice(i * P, (i + 1) * P)
                gath = state[i]["gath"]
                gv = gath.rearrange("p (k c) -> p k c", k=KNN)
                av = apm[:, i * WD:(i + 1) * WD] \
                    .rearrange("p (o c) -> p o c", o=1) \
                    .to_broadcast([128, KNN, WD])
                nc.vector.tensor_add(gv, gv, av)
                nc.scalar.activation(gath[:, :], gath[:, :], ACT.Relu)

                z1T = sp.tile([H, KNN * 128], ED, name="z1T", tag="z1T", bufs=2)
                for c in range(5):
                    cs = slice(c * 512, (c + 1) * 512)
                    ptr = spp.tile([H, 512], ED, name="ptr", tag="ptr", bufs=1,
                                   space="PSUM")
                    for m in range(4):
                        k = c * 4 + m
                        nc.tensor.transpose(
                            ptr[:, m * 128:(m + 1) * 128],
                            gath[:, k * WD:(k + 1) * WD],
                            idt[:, :])
                    nc.scalar.copy(W(z1T[:, cs]), ptr[:, :])
                # ---- layer 2 ----
                z2T = sp.tile([H, KNN * 128], ED, name="z2T", tag="z2T", bufs=2)
                for c in range(5):
                    cs = slice(c * 512, (c + 1) * 512)
                    pm = spp.tile([H, 512], f32, name="pm", tag="pm", bufs=1,
                                  space="PSUM")
                    nc.tensor.matmul(pm[:], W(wl2[:, :]), W(z1T[:, cs]))
                    nc.scalar.activation(W(z2T[:, cs]), pm[:], ACT.Relu,
                                         bias=bl2[:, 0:1])
                # ---- layer 3, half 0: all chunks into one 5-bank PSUM tile
                pl = spp.tile([H, 5 * 512], f32, name="pl", tag="pl", bufs=1,
                              space="PSUM")
                for c in range(5):
                    wsel = wl3[:, :] if conv == 1 else wl3[:, 0:128]
                    nc.tensor.matmul(pl[:, c * 512:(c + 1) * 512],
                                     W(wsel), W(z2T[:, c * 512:(c + 1) * 512]))
                state[i].update(z2T=z2T, pl=pl)

            def stage_mlp_b(i):
                # max over K for half 0 (one strided reduce over 5 banks),
                # then L3 matmuls for half 1 (conv2) reusing the same banks.
                isl = slice(i * P, (i + 1) * P)
                pl = state[i]["pl"]
                redf = sp.tile([128, 128], f32, name="redf", tag="redf", bufs=2)
                nc.vector.tensor_reduce(
                    redf[0:H, :],
                    pl[:, :].rearrange("p (c k n) -> p n (c k)", c=5, k=4),
                    axis=AX.X, op=OP.max)
                if conv == 1:
                    nc.scalar.activation(A2[0:64, isl], redf[0:64, :],
                                         ACT.Relu, bias=bl3[:, 0:1])
                else:
                    nc.scalar.activation(mmo(x2Ta[:, isl]), redf[:, :],
                                         ACT.Relu, bias=bl3[:, 0:1])
                    z2T = state[i]["z2T"]
                    pl2 = spp.tile([H, 5 * 512], f32, name="pl2", tag="pl",
                                   bufs=1, space="PSUM")
                    for c in range(5):
                        nc.tensor.matmul(
                            pl2[:, c * 512:(c + 1) * 512],
                            wl3[:, 128:256], W(z2T[:, c * 512:(c + 1) * 512]))
                    state[i]["pl2"] = pl2

            def stage_mlp_c(i):
                if conv == 1:
                    del state[i]
                    return
                isl = slice(i * P, (i + 1) * P)
                pl2 = state[i]["pl2"]
                redf = sp.tile([128, 128], f32, name="redf", tag="redf", bufs=2)
                nc.vector.tensor_reduce(
                    redf[:, :],
                    pl2[:, :].rearrange("p (c k n) -> p n (c k)", c=5, k=4),
                    axis=AX.X, op=OP.max)
                nc.scalar.activation(mmo(x2Tb[:, isl]), redf[:, :],
                                     ACT.Relu, bias=bl3[:, 1:2])
                del state[i]

            # software pipeline: tile i+1's scores overlap tile i's reduces,
            # and tile i+1's DVE top-k rounds run while tile i's half-1
            # matmuls and reduce drain.
            stage_scores(0)
            stage_topk(0)
            for i in range(NT):
                stage_mlp_a(i)
                if i + 1 < NT:
                    stage_scores(i + 1)
                stage_mlp_b(i)
                if i + 1 < NT:
                    stage_topk(i + 1)
                stage_mlp_c(i)

        # =============== conv1 ===============
        with tc.tile_pool(name="c1", bufs=2) as sp, \
             tc.tile_pool(name="c1ps", bufs=2, space="PSUM") as spp:
            edge_conv(1, sp, spp)

        # =============== conv2 prep ===============
        with tc.tile_pool(name="prep2", bufs=2) as pp, \
             tc.tile_pool(name="prep2_ps", bufs=2, space="PSUM") as ppp:
            # s2 row + s2 columns (engines cannot shift partitions: stage the
            # row at partition 0, DMA it into B2 row 64)
            nc.scalar.activation(vscratch[0:64, :], A2[0:64, :], ACT.Square)
            ones64 = g.tile([64, 1], f32, name="ones64")
            nc.vector.memset(ones64[:, :], 1.0)
            s2tmp = pp.tile([1, N], f32, name="s2tmp")
            for c in range(4):
                cs = slice(c * 512, (c + 1) * 512)
                ps2 = ppp.tile([1, 512], f32, name="ps2", space="PSUM", bufs=1)
                nc.tensor.matmul(ps2[:], ones64[:, :], vscratch[0:64, cs])
                nc.scalar.copy(s2tmp[0:1, cs], ps2[:])
            nc.sync.dma_start(B2[64:65, :], s2tmp[:, :])
            for i in range(NT):
                isl = slice(i * P, (i + 1) * P)
                tsc = ppp.tile([128, 1], f32, name="tsc", space="PSUM", bufs=1)
                nc.tensor.transpose(tsc[:], s2tmp[0:1, isl], ident_sb[0:1, 0:1])
                nc.scalar.mul(nscol2[:, i:i + 1], tsc[:], -1.0)
            nc.scalar.mul(B2[0:64, :], A2[0:64, :], -2.0)
            # a_i point-major (A2 rows carry [x1;1], W2comb = [W2a-W2b; b2])
            for i in range(NT):
                isl = slice(i * P, (i + 1) * P)
                pa2 = ppp.tile([128, 128], f32, name="pa2", space="PSUM")
                nc.tensor.matmul(pa2[:], A2[0:65, isl], W2comb[:, :])
                nc.scalar.copy(apm2[:, i * 128:(i + 1) * 128], pa2[:])
            # v2 = x1 @ W2b -> DRAM rows
            for c in range(4):
                cs = slice(c * 512, (c + 1) * 512)
                pv = ppp.tile([128, 512], f32, name="pv2", space="PSUM")
                nc.tensor.matmul(pv[:], w_c2w1b[:, :], A2[0:64, cs])
                nc.scalar.copy(vscratch[:, cs], pv[:])
            for grp in range(4):
                vstage = pp.tile([128, 512], bf16, name="vstage2")
                for m in range(4):
                    i = grp * 4 + m
                    tvp = ppp.tile([128, 128], f32, name="tvp2", space="PSUM")
                    nc.tensor.transpose(tvp[:], vscratch[:, i * P:(i + 1) * P],
                                        ident_sb[:, :])
                    nc.vector.tensor_copy(vstage[:, m * 128:(m + 1) * 128], tvp[:])
                nc.sync.dma_start(
                    v2d[:, :].rearrange("(g m r) ch -> g r m ch", g=4, m=4)[grp],
                    vstage[:, :])

        # =============== conv2 ===============
        with tc.tile_pool(name="c2", bufs=2) as sp, \
             tc.tile_pool(name="c2ps", bufs=2, space="PSUM") as spp:
            edge_conv(2, sp, spp)

        # =============== classifier ===============
        with tc.tile_pool(name="cls", bufs=2) as cp, \
             tc.tile_pool(name="clsps", bufs=2, space="PSUM") as cpp:
            pooled = g.tile([128, 4], f32, name="pooled")
            for t_ in range(4):
                tsl = slice(t_ * 128, (t_ + 1) * 128)
                ps = cpp.tile([128, 2048], f32, name="ps_l0", tag="ps_l0", bufs=1)
                for c in range(4):
                    cs = slice(c * 512, (c + 1) * 512)
                    nc.tensor.matmul(ps[:, cs], mm(w_l0w[:, 0:512][:, tsl]),
                                     mm(x2Ta[:, cs]), start=True, stop=False)
                    nc.tensor.matmul(ps[:, cs], mm(w_l0w[:, 512:1024][:, tsl]),
                                     mm(x2Tb[:, cs]), start=False, stop=True)
                pool1 = cp.tile([128, 1], f32, name="pool1")
                nc.vector.tensor_reduce(pool1[:, :], ps[:, :], axis=AX.X, op=OP.max)
                nc.scalar.activation(pooled[:, t_:t_ + 1], pool1[:, :],
                                     ACT.Relu, bias=b_l0b[:, t_:t_ + 1])
            # l1: 512 -> 256
            y1 = g.tile([128, 2], f32, name="y1")
            for h in range(2):
                ps1 = cpp.tile([128, 1], f32, name="ps_l1", tag="ps_s")
                for c in range(4):
                    nc.tensor.matmul(ps1[:],
                                     w_l1w[:, c * 256 + h * 128: c * 256 + (h + 1) * 128].bitcast(f32),
                                     pooled[:, c:c + 1],
                                     start=(c == 0), stop=(c == 3))
                nc.scalar.activation(y1[:, h:h + 1], ps1[:, :], ACT.Relu,
                                     bias=b_l1b[:, h:h + 1])
            # l2: 256 -> 256
            y2 = g.tile([128, 2], f32, name="y2")
            for h in range(2):
                ps2_ = cpp.tile([128, 1], f32, name="ps_l2", tag="ps_s")
                for c in range(2):
                    nc.tensor.matmul(ps2_[:],
                                     w_l2w[:, c * 256 + h * 128: c * 256 + (h + 1) * 128].bitcast(f32),
                                     y1[:, c:c + 1],
                                     start=(c == 0), stop=(c == 1))
                nc.scalar.activation(y2[:, h:h + 1], ps2_[:, :], ACT.Relu,
                                     bias=b_l2b[:, h:h + 1])
            # l3: 256 -> 40
            ps3 = cpp.tile([NUM_CLASSES, 1], f32, name="ps_l3", tag="ps_s")
            for c in range(2):
                nc.tensor.matmul(ps3[:],
                                 w_l3w[:, c * NUM_CLASSES:(c + 1) * NUM_CLASSES].bitcast(f32),
                                 y2[:, c:c + 1],
                                 start=(c == 0), stop=(c == 1))
            y3 = cp.tile([NUM_CLASSES, 1], f32, name="y3")
            nc.vector.tensor_add(y3[:, :], ps3[:, :], b_l3b[:, :])
            # log_softmax over the 40 values: transpose to one row
            pr = cpp.tile([1, NUM_CLASSES], f32, name="pr", tag="ps_s")
            nc.tensor.transpose(pr[:], y3[:, :], ident_sb[0:NUM_CLASSES, 0:NUM_CLASSES])
            row = cp.tile([1, NUM_CLASSES], f32, name="row")
            nc.vector.tensor_copy(row[:, :], pr[:, :])
            mx = cp.tile([1, 1], f32, name="mx")
            nc.vector.tensor_reduce(mx[:, :], row[:, :], axis=AX.X, op=OP.max)
            nmx = cp.tile([1, 1], f32, name="nmx")
            nc.scalar.mul(nmx[:, :], mx[:, :], -1.0)
            ex = cp.tile([1, NUM_CLASSES], f32, name="ex")
            sacc = cp.tile([1, 1], f32, name="sacc")
            nc.scalar.activation(ex[:, :], row[:, :], ACT.Exp,
                                 bias=nmx[:, 0:1], accum_out=sacc[:, :])
            lnz = cp.tile([1, 1], f32, name="lnz")
            nc.scalar.activation(lnz[:, :], sacc[:, :], ACT.Ln)
            shift = cp.tile([1, 1], f32, name="shift")
            nc.vector.tensor_sub(shift[:, :], lnz[:, :], nmx[:, :])
            osb = cp.tile([1, NUM_CLASSES], f32, name="osb")
            nc.vector.tensor_scalar(osb[:, :], row[:, :], shift[:, 0:1],
                                    None, op0=OP.subtract)
            nc.sync.dma_start(out[:, :], osb[:, :])

        ctx.close()

    nc.compile()
    return nc


def _get_program():
    if "nc" not in _PROGRAM_CACHE:
        _PROGRAM_CACHE["nc"] = _build_program()
    return _PROGRAM_CACHE["nc"]


def _in_maps(inputs):
    w_names = ["c1w1", "c1b1", "c1w2", "c1b2", "c1w3", "c1b3",
               "c2w1", "c2b1", "c2w2", "c2b2", "c2w3", "c2b3",
               "l0w", "l0b", "l1w", "l1b", "l2w", "l2b", "l3w", "l3b"]
    shared = {k: np.ascontiguousarray(np.asarray(inputs[k], np.float32))
              for k in w_names}
    shared["ident"] = np.eye(128, dtype=np.float32)
    pos = np.ascontiguousarray(np.asarray(inputs["pos"], np.float32))
    maps = []
    for c in range(NCLOUD):
        m = dict(shared)
        m["pos"] = np.ascontiguousarray(pos[c * N:(c + 1) * N])
        maps.append(m)
    return maps


def kernel(**inputs) -> np.ndarray:
    from concourse import bass_utils
    nc = _get_program()
    maps = _in_maps(inputs)
    res = bass_utils.run_bass_kernel_spmd(nc, maps, core_ids=list(range(NCLOUD)))
    outs = [np.asarray(r["out"]).reshape(1, NUM_CLASSES) for r in res.results]
    return np.concatenate(outs, axis=0).astype(np.float32)


# revision 6
# speedup vs baseline: 1.0136x; 1.0136x over previous
"""DGCNN (dynamic edge conv x2 + classifier) Trainium2 Bass kernel.

Sharding: data-parallel over the 8 point clouds -> 8 NeuronCores.
Each core runs the full per-cloud pipeline:
  conv1: kNN in 3-D, edge MLP 6->64->64->64, max over K=20
  conv2: kNN in 64-D feature space, edge MLP 128->128->128->256, max over K
  head : 256->512, global max pool, 512->256->256->40, log_softmax

Key per-core implementation ideas:
  * kNN scores via one augmented matmul: [x,1] @ [-2x; |x|^2]^T.
  * composite sort keys: each u32 word = fp16(-d2) in the high 16 bits
    (written by the scalar engine straight from PSUM with a strided AP)
    and a persistent u16 column-index iota in the low 16 bits.  Read as
    f32, lexicographic float order ranks by distance with deterministic
    index tie-breaks, so DVE max8 alone yields both value and index --
    no max_index scans, and neighbor indices pop out with a bitwise AND.
  * top-24 via 3 rounds of max8; match_replace writes into a scratch
    copy so the key tile (and its iota) is never clobbered.
  * per-round batched indirect-DMA gathers (8 offsets per call) instead
    of one call per neighbor: 3 GPSIMD descriptor-gen ops per tile.
  * edge first layer decomposed: z1 = relu(a_i + v_j) with
    a = x@(W1a-W1b)+b1 (point-major, one small matmul per tile) and
    v = x@W1b gathered from DRAM; the broadcast add runs point-major as
    a single DVE op over all 20 neighbors before the PE transposes.
  * layer-3 outputs for a whole tile land in one multi-bank PSUM tile;
    the max over K collapses to one strided tensor_reduce per half.
"""

import os
import sys
import numpy as np

for _p in ("/opt/trn_rl_repo",):
    if _p not in sys.path:
        sys.path.insert(0, _p)

N = 2048          # points per cloud
NCLOUD = 8
P = 128           # partition tile
NT = N // P       # 16 row tiles
KNN = 20
KSEL = 24         # 3 rounds x 8
NEG_BIG = -3.0e38
NUM_CLASSES = 40

# matmul dtype knobs: None -> plain float32; "f32r" -> float32r fast path
MM_FAST_MLP = True     # edge-MLP layers 2/3 + classifier matmuls
MM_FAST_SCORES = False  # kNN score matmuls (selection-critical, keep exact)

_PROGRAM_CACHE = {}


def _build_program():
    import concourse.bass as bass
    import concourse.bacc as bacc
    import concourse.tile as tile
    from concourse import mybir

    f32 = mybir.dt.float32
    f32r = mybir.dt.float32r
    f16 = mybir.dt.float16
    bf16 = mybir.dt.bfloat16
    u16 = mybir.dt.uint16
    u32 = mybir.dt.uint32
    AX = mybir.AxisListType
    OP = mybir.AluOpType
    ACT = mybir.ActivationFunctionType

    def mm(ap):
        return ap.bitcast(f32r) if MM_FAST_MLP else ap

    def mms(ap):
        return ap.bitcast(f32r) if MM_FAST_SCORES else ap

    mmo = mm  # producer outputs feeding fast matmuls must round to f32r

    nc = bacc.Bacc("TRN2", target_bir_lowering=False, debug=False)

    # ---------------- I/O ----------------
    def din(name, shape):
        return nc.dram_tensor(name, list(shape), f32, kind="ExternalInput").ap()

    pos = din("pos", [N, 3])
    c1w1 = din("c1w1", [6, 64]);   c1b1 = din("c1b1", [64])
    c1w2 = din("c1w2", [64, 64]);  c1b2 = din("c1b2", [64])
    c1w3 = din("c1w3", [64, 64]);  c1b3 = din("c1b3", [64])
    c2w1 = din("c2w1", [128, 128]); c2b1 = din("c2b1", [128])
    c2w2 = din("c2w2", [128, 128]); c2b2 = din("c2b2", [128])
    c2w3 = din("c2w3", [128, 256]); c2b3 = din("c2b3", [256])
    l0w = din("l0w", [256, 512]);  l0b = din("l0b", [512])
    l1w = din("l1w", [512, 256]);  l1b = din("l1b", [256])
    l2w = din("l2w", [256, 256]);  l2b = din("l2b", [256])
    l3w = din("l3w", [256, NUM_CLASSES]); l3b = din("l3b", [NUM_CLASSES])
    ident = din("ident", [128, 128])

    out = nc.dram_tensor("out", [1, NUM_CLASSES], f32, kind="ExternalOutput").ap()

    with tile.TileContext(nc) as tc:
        from contextlib import ExitStack

        ctx = ExitStack()
        g = ctx.enter_context(tc.tile_pool(name="g", bufs=1))          # persistent
        dpool = ctx.enter_context(tc.tile_pool(name="dram", bufs=1, space="DRAM"))

        # persistent SBUF state
        ident_sb = g.tile([128, 128], f32)
        nc.sync.dma_start(ident_sb[:], ident[:, :])
        ident_bf = g.tile([128, 128], bf16)
        nc.scalar.copy(ident_bf[:, :], ident_sb[:, :])

        A1 = g.tile([4, N], f32)       # [x^T ; 1]
        B1 = g.tile([4, N], f32)       # [-2 x^T ; s]
        A2 = g.tile([65, N], f32)      # [x1^T ; 1]
        B2 = g.tile([65, N], f32)      # [-2 x1^T ; s2]
        x2Ta = g.tile([128, N], f32)   # conv2 out ch 0:128
        x2Tb = g.tile([128, N], f32)   # conv2 out ch 128:256
        nscol1 = g.tile([128, NT], f32)  # -s_i per tile column
        nscol2 = g.tile([128, NT], f32)
        vscratch = g.tile([128, N], f32, name="vscratch")  # v1T/x1sq/v2T staging
        apm1 = g.tile([128, NT * 64], f32, name="apm1")    # a_i point-major
        apm2 = g.tile([128, NT * 128], bf16, name="apm2")

        # composite-key ring: u32 word = fp16(-d2) << 16 | column index.
        # The u16 iota in the low halves persists across tiles and convs;
        # only the fp16 halves are rewritten (strided ACT store from PSUM).
        KB = 3
        keybufs = [g.tile([128, 2 * N], u16, name=f"key{r}") for r in range(KB)]
        for kb in keybufs:
            ev = kb.rearrange("p (n two) -> p n two", two=2)[:, :, 0:1]
            nc.gpsimd.iota(ev, pattern=[[1, N]], base=0, channel_multiplier=0)

        v1d = dpool.tile([N, 64], f32, name="v1d")
        v2d = dpool.tile([N, 128], bf16, name="v2d")

        # weights / biases.  Weights consumed by f32r matmuls are loaded into
        # a scratch tile and rounded into an f32r-typed tile with an ACT copy
        # (the BIR verifier requires every writer of an f32r matmul operand
        # to emit rounded data, so the DMA cannot write them directly).
        wraw = ctx.enter_context(tc.tile_pool(name="wraw", bufs=2))

        def load_w(name, shape, pieces, wdt=None):
            wdt = wdt or (f32r if MM_FAST_MLP else f32)
            if wdt != f32:
                raw = wraw.tile(list(shape), f32, name=name + "_raw", tag="wraw")
                for sl, srcap in pieces:
                    nc.sync.dma_start(raw[sl], srcap)
                t = g.tile(list(shape), wdt, name=name)
                nc.scalar.copy(t[:, :], raw[:, :])
            else:
                t = g.tile(list(shape), f32, name=name)
                for sl, srcap in pieces:
                    nc.sync.dma_start(t[sl], srcap)
            return t

        SALL = (slice(None), slice(None))
        w_c1w1a = g.tile([3, 64], f32); nc.sync.dma_start(w_c1w1a[:], c1w1[0:3, :])
        w_c1w1b = g.tile([3, 64], f32); nc.sync.dma_start(w_c1w1b[:], c1w1[3:6, :])
        w_c2w1a = g.tile([64, 128], f32); nc.sync.dma_start(w_c2w1a[:], c2w1[0:64, :])
        w_c2w1b = g.tile([64, 128], f32); nc.sync.dma_start(w_c2w1b[:], c2w1[64:128, :])
        w_c1w2 = load_w("w_c1w2", [64, 64], [(SALL, c1w2[:, :])])
        w_c1w3 = load_w("w_c1w3", [64, 64], [(SALL, c1w3[:, :])])
        w_c2w2 = load_w("w_c2w2", [128, 128], [(SALL, c2w2[:, :])], wdt=bf16)
        w_c2w3 = load_w("w_c2w3", [128, 256], [(SALL, c2w3[:, :])], wdt=bf16)
        w_l0w = load_w("w_l0w", [128, 1024],
                       [((slice(None), slice(0, 512)), l0w[0:128, :]),
                        ((slice(None), slice(512, 1024)), l0w[128:256, :])])
        w_l1w = load_w("w_l1w", [128, 1024],
                       [((slice(None), slice(c * 256, (c + 1) * 256)),
                         l1w[c * 128:(c + 1) * 128, :]) for c in range(4)])
        w_l2w = load_w("w_l2w", [128, 512],
                       [((slice(None), slice(0, 256)), l2w[0:128, :]),
                        ((slice(None), slice(256, 512)), l2w[128:256, :])])
        w_l3w = load_w("w_l3w", [128, 2 * NUM_CLASSES],
                       [((slice(None), slice(0, NUM_CLASSES)), l3w[0:128, :]),
                        ((slice(None), slice(NUM_CLASSES, 2 * NUM_CLASSES)), l3w[128:256, :])])

        # first-layer combined weights: a = x @ (W1a - W1b) + b  (point-major)
        W1comb = g.tile([4, 64], f32, name="W1comb")
        nc.vector.tensor_sub(W1comb[0:3, :], w_c1w1a[:, :], w_c1w1b[:, :])
        nc.sync.dma_start(W1comb[3:4, :], c1b1.rearrange("(o c) -> o c", o=1))
        W2comb = g.tile([65, 128], f32, name="W2comb")
        nc.vector.tensor_sub(W2comb[0:64, :], w_c2w1a[:, :], w_c2w1b[:, :])
        nc.sync.dma_start(W2comb[64:65, :], c2b1.rearrange("(o c) -> o c", o=1))

        def col(name, src, n):
            t = g.tile([n, 1], f32, name=name)
            nc.sync.dma_start(t[:, :], src.rearrange("(c o) -> c o", o=1))
            return t

        b_c1b2 = col("b_c1b2", c1b2, 64)
        b_c1b3 = col("b_c1b3", c1b3, 64)
        b_c2b2 = col("b_c2b2", c2b2, 128)
        b_c2b3 = g.tile([128, 2], f32)
        nc.sync.dma_start(b_c2b3[:, 0:1], c2b3.rearrange("(h c o) -> h c o", h=2, o=1)[0])
        nc.sync.dma_start(b_c2b3[:, 1:2], c2b3.rearrange("(h c o) -> h c o", h=2, o=1)[1])
        b_l0b = g.tile([128, 4], f32)
        for t_ in range(4):
            nc.sync.dma_start(b_l0b[:, t_:t_ + 1],
                              l0b.rearrange("(h c o) -> h c o", h=4, o=1)[t_])
        b_l1b = g.tile([128, 2], f32)
        for t_ in range(2):
            nc.sync.dma_start(b_l1b[:, t_:t_ + 1],
                              l1b.rearrange("(h c o) -> h c o", h=2, o=1)[t_])
        b_l2b = g.tile([128, 2], f32)
        for t_ in range(2):
            nc.sync.dma_start(b_l2b[:, t_:t_ + 1],
                              l2b.rearrange("(h c o) -> h c o", h=2, o=1)[t_])
        b_l3b = col("b_l3b", l3b, NUM_CLASSES)

        # engines cannot address partition bases 3/64 directly: stage a ones
        # row at partition 0 and DMA it into place
        ones_row = g.tile([1, N], f32, name="ones_row")
        nc.vector.memset(ones_row[:, :], 1.0)
        nc.sync.dma_start(A1[3:4, :], ones_row[:, :])
        nc.sync.dma_start(A2[64:65, :], ones_row[:, :])

        # =============== conv1 prep ===============
        with tc.tile_pool(name="prep", bufs=2) as pp, \
             tc.tile_pool(name="prep_ps", bufs=2, space="PSUM") as ppp:
            scol = g.tile([128, NT], f32, name="scol1_pos")
            for i in range(NT):
                isl = slice(i * P, (i + 1) * P)
                pt = pp.tile([128, 3], f32, name="pt")
                nc.sync.dma_start(pt[:], pos[isl, :])
                sq = pp.tile([128, 3], f32, name="sq")
                nc.scalar.activation(sq[:], pt[:], ACT.Square,
                                     accum_out=scol[:, i:i + 1])
                tp = ppp.tile([3, 128], f32, name="tp", space="PSUM", bufs=1)
                nc.tensor.transpose(tp[:], pt[:], ident_sb[:])
                nc.scalar.copy(A1[0:3, isl], tp[:])
            nc.scalar.mul(nscol1[:, :], scol[:, :], -1.0)
            nc.scalar.mul(B1[0:3, :], A1[0:3, :], -2.0)
            # s row: transpose scol [128, NT] -> [NT, 128], stage in SBUF, then
            # one cross-partition DMA into B1 row 3 (engines cannot shift
            # partitions; DMA can).
            stp = ppp.tile([NT, 128], f32, name="stp", space="PSUM", bufs=1)
            nc.tensor.transpose(stp[:], scol[:, :], ident_sb[:])
            srow_sb = pp.tile([NT, 128], f32, name="srow_sb")
            nc.scalar.copy(srow_sb[:, :], stp[:, :])
            nc.sync.dma_start(
                B1[3:4, :].rearrange("o (p n) -> o p n", p=NT), srow_sb[:, :])

            # a_i point-major: one small matmul per tile (A1 rows carry [x;1])
            for i in range(NT):
                isl = slice(i * P, (i + 1) * P)
                pa = ppp.tile([128, 64], f32, name="pa", space="PSUM")
                nc.tensor.matmul(pa[:], A1[0:4, isl], W1comb[:, :])
                nc.scalar.copy(apm1[:, i * 64:(i + 1) * 64], pa[:])

            # v1 = x @ W1b, channel-major; rows -> DRAM [N, 64]
            for c in range(4):
                cs = slice(c * 512, (c + 1) * 512)
                pv = ppp.tile([64, 512], f32, name="pv", space="PSUM")
                nc.tensor.matmul(pv[:], w_c1w1b[:, :], A1[0:3, cs])
                nc.scalar.copy(vscratch[0:64, cs], pv[:])
            for grp in range(4):
                vstage = pp.tile([128, 256], f32, name="vstage")
                for m in range(4):
                    i = grp * 4 + m
                    tvp = ppp.tile([128, 64], f32, name="tvp", space="PSUM")
                    nc.tensor.transpose(tvp[:], vscratch[0:64, i * P:(i + 1) * P],
                                        ident_sb[0:64, 0:64])
                    nc.vector.tensor_copy(vstage[:, m * 64:(m + 1) * 64], tvp[:])
                nc.sync.dma_start(
                    v1d[:, :].rearrange("(g m r) ch -> g r m ch", g=4, m=4)[grp],
                    vstage[:, :])

        # =============== edge-conv block (shared structure) ===============
        def edge_conv(conv, sp, spp):
            """conv=1: H=64 channels; conv=2: H=128 (256 out)."""
            if conv == 1:
                H, CON, WD, ED, idt = 64, 4, 64, f32, ident_sb
                Asb, Bsb, vd, nscol, apm = A1, B1, v1d, nscol1, apm1
                wl2, wl3 = w_c1w2, w_c1w3
                bl2, bl3 = b_c1b2, b_c1b3
                W = mm        # f32r bitcast for conv1 MLP
            else:
                H, CON, WD, ED, idt = 128, 65, 128, bf16, ident_bf
                Asb, Bsb, vd, nscol, apm = A2, B2, v2d, nscol2, apm2
                wl2, wl3 = w_c2w2, w_c2w3
                bl2, bl3 = b_c2b2, b_c2b3
                W = lambda ap: ap   # tiles already bf16
            nhalf = 1 if conv == 1 else 2

            state = {}

            def stage_scores(i):
                isl = slice(i * P, (i + 1) * P)
                key = keybufs[i % KB]
                keyh = key.bitcast(f16).rearrange("p (n two) -> p n two", two=2)
                for c in range(4):
                    cs = slice(c * 512, (c + 1) * 512)
                    psc = spp.tile([128, 512], f32, name="psc", tag="psc", bufs=1)
                    nc.tensor.matmul(psc[:, :],
                                     mms(Asb[0:CON, isl]), mms(Bsb[0:CON, cs]))
                    # fp16(-d2) into the high u16 halves (strided store)
                    nc.scalar.activation(
                        keyh[:, cs, 1:2],
                        psc[:, :].rearrange("p (n o) -> p n o", o=1),
                        ACT.Identity, bias=nscol[:, i:i + 1], scale=-1.0)
                state[i] = {"key": key}

            def stage_topk(i):
                # 3 rounds of max8 over the composite keys; match_replace
                # writes into a scratch copy so the key iota survives.  The
                # neighbor index is the low 16 bits of each winning key.
                key32 = state[i]["key"].bitcast(f32)
                vals = sp.tile([128, KSEL], u32, name="vals", tag="vals", bufs=4)
                valsf = vals.bitcast(f32)
                idx = sp.tile([128, KSEL], u32, name="idx", tag="idx", bufs=4)
                scr = sp.tile([128, N], f32, name="scr", tag="scr", bufs=2)
                for r in range(3):
                    rs = slice(r * 8, (r + 1) * 8)
                    src = key32 if r == 0 else scr[:, :]
                    nc.vector.max(valsf[:, rs], src)
                    if r < 2:
                        nc.vector.match_replace(scr[:, :], valsf[:, rs], src,
                                                NEG_BIG)
                    nc.vector.tensor_scalar(idx[:, rs], vals[:, rs],
                                            0x7FF, None, op0=OP.bitwise_and)
                state[i].update(idx=idx)

            def stage_gather(i):
                # HW indirect DMA consumes ONE offset per destination
                # partition, so issue one gather per neighbor slot.  Runs a
                # full pipeline stage ahead of the consumer so the ~1.1us
                # per-call GPSIMD descriptor-gen cost overlaps tile i-1's
                # MLP instead of stalling it.
                idx = state[i]["idx"]
                gath = sp.tile([128, KNN * WD], ED, name="gath", tag="gath", bufs=3)
                for k in range(KNN):
                    nc.gpsimd.indirect_dma_start(
                        out=gath[:, k * WD:(k + 1) * WD], out_offset=None,
                        in_=vd[:, :],
                        in_offset=bass.IndirectOffsetOnAxis(
                            ap=idx[:, k:k + 1], axis=0),
                    )
                state[i].update(gath=gath)

            def stage_mlp_a(i):
                # z1 = relu(a_i + v_j) point-major, then PE transposes to
                # channel-major; L2; L3 matmuls for half 0.
                isl = slice(i * P, (i + 1) * P)
                gath = state[i]["gath"]
                gv = gath.rearrange("p (k c) -> p k c", k=KNN)
                av = apm[:, i * WD:(i + 1) * WD] \
                    .rearrange("p (o c) -> p o c", o=1) \
                    .to_broadcast([128, KNN, WD])
                nc.vector.tensor_add(gv, gv, av)
                nc.scalar.activation(gath[:, :], gath[:, :], ACT.Relu)

                z1T = sp.tile([H, KNN * 128], ED, name="z1T", tag="z1T", bufs=2)
                for c in range(5):
                    cs = slice(c * 512, (c + 1) * 512)
                    ptr = spp.tile([H, 512], ED, name="ptr", tag="ptr", bufs=1,
                                   space="PSUM")
                    for m in range(4):
                        k = c * 4 + m
                        nc.tensor.transpose(
                            ptr[:, m * 128:(m + 1) * 128],
                            gath[:, k * WD:(k + 1) * WD],
                            idt[:, :])
                    nc.scalar.copy(W(z1T[:, cs]), ptr[:, :])
                # ---- layer 2 ----
                z2T = sp.tile([H, KNN * 128], ED, name="z2T", tag="z2T", bufs=2)
                for c in range(5):
                    cs = slice(c * 512, (c + 1) * 512)
                    pm = spp.tile([H, 512], f32, name="pm", tag="pm", bufs=1,
                                  space="PSUM")
                    nc.tensor.matmul(pm[:], W(wl2[:, :]), W(z1T[:, cs]))
                    nc.scalar.activation(W(z2T[:, cs]), pm[:], ACT.Relu,
                                         bias=bl2[:, 0:1])
                # ---- layer 3, half 0: all chunks into one 5-bank PSUM tile
                pl = spp.tile([H, 5 * 512], f32, name="pl", tag="pl", bufs=1,
                              space="PSUM")
                for c in range(5):
                    wsel = wl3[:, :] if conv == 1 else wl3[:, 0:128]
                    nc.tensor.matmul(pl[:, c * 512:(c + 1) * 512],
                                     W(wsel), W(z2T[:, c * 512:(c + 1) * 512]))
                state[i].update(z2T=z2T, pl=pl)

            def stage_mlp_b(i):
                # max over K for half 0 (one strided reduce over 5 banks),
                # then L3 matmuls for half 1 (conv2) reusing the same banks.
                isl = slice(i * P, (i + 1) * P)
                pl = state[i]["pl"]
                redf = sp.tile([128, 128], f32, name="redf", tag="redf", bufs=2)
                nc.vector.tensor_reduce(
                    redf[0:H, :],
                    pl[:, :].rearrange("p (c k n) -> p n (c k)", c=5, k=4),
                    axis=AX.X, op=OP.max)
                if conv == 1:
                    nc.scalar.activation(A2[0:64, isl], redf[0:64, :],
                                         ACT.Relu, bias=bl3[:, 0:1])
                else:
                    nc.scalar.activation(mmo(x2Ta[:, isl]), redf[:, :],
                                         ACT.Relu, bias=bl3[:, 0:1])
                    z2T = state[i]["z2T"]
                    pl2 = spp.tile([H, 5 * 512], f32, name="pl2", tag="pl",
                                   bufs=1, space="PSUM")
                    for c in range(5):
                        nc.tensor.matmul(
                            pl2[:, c * 512:(c + 1) * 512],
                            wl3[:, 128:256], W(z2T[:, c * 512:(c + 1) * 512]))
                    state[i]["pl2"] = pl2

            def stage_mlp_c(i):
                if conv == 1:
                    del state[i]
                    return
                isl = slice(i * P, (i + 1) * P)
                pl2 = state[i]["pl2"]
                redf = sp.tile([128, 128], f32, name="redf", tag="redf", bufs=2)
                nc.vector.tensor_reduce(
                    redf[:, :],
                    pl2[:, :].rearrange("p (c k n) -> p n (c k)", c=5, k=4),
                    axis=AX.X, op=OP.max)
                nc.scalar.activation(mmo(x2Tb[:, isl]), redf[:, :],
                                     ACT.Relu, bias=bl3[:, 1:2])
                del state[i]

            # 3-deep software pipeline: while tile i's MLP runs, the GPSIMD
            # queue drains tile i+1's 20 gathers (issued a full stage after
            # their top-k) and the DVE runs tile i+2's top-k rounds, so no
            # engine waits on same-tile producers.
            stage_scores(0)
            stage_topk(0)
            stage_gather(0)
            if NT > 1:
                stage_scores(1)
                stage_topk(1)
            for i in range(NT):
                stage_mlp_a(i)
                if i + 2 < NT:
                    stage_scores(i + 2)
                if i + 1 < NT:
                    stage_gather(i + 1)
                stage_mlp_b(i)
                if i + 2 < NT:
                    stage_topk(i + 2)
                stage_mlp_c(i)

        # =============== conv1 ===============
        with tc.tile_pool(name="c1", bufs=2) as sp, \
             tc.tile_pool(name="c1ps", bufs=2, space="PSUM") as spp:
            edge_conv(1, sp, spp)

        # =============== conv2 prep ===============
        with tc.tile_pool(name="prep2", bufs=2) as pp, \
             tc.tile_pool(name="prep2_ps", bufs=2, space="PSUM") as ppp:
            # s2 row + s2 columns (engines cannot shift partitions: stage the
            # row at partition 0, DMA it into B2 row 64)
            nc.scalar.activation(vscratch[0:64, :], A2[0:64, :], ACT.Square)
            ones64 = g.tile([64, 1], f32, name="ones64")
            nc.vector.memset(ones64[:, :], 1.0)
            s2tmp = pp.tile([1, N], f32, name="s2tmp")
            for c in range(4):
                cs = slice(c * 512, (c + 1) * 512)
                ps2 = ppp.tile([1, 512], f32, name="ps2", space="PSUM", bufs=1)
                nc.tensor.matmul(ps2[:], ones64[:, :], vscratch[0:64, cs])
                nc.scalar.copy(s2tmp[0:1, cs], ps2[:])
            nc.sync.dma_start(B2[64:65, :], s2tmp[:, :])
            for i in range(NT):
                isl = slice(i * P, (i + 1) * P)
                tsc = ppp.tile([128, 1], f32, name="tsc", space="PSUM", bufs=1)
                nc.tensor.transpose(tsc[:], s2tmp[0:1, isl], ident_sb[0:1, 0:1])
                nc.scalar.mul(nscol2[:, i:i + 1], tsc[:], -1.0)
            nc.scalar.mul(B2[0:64, :], A2[0:64, :], -2.0)
            # a_i point-major (A2 rows carry [x1;1], W2comb = [W2a-W2b; b2])
            for i in range(NT):
                isl = slice(i * P, (i + 1) * P)
                pa2 = ppp.tile([128, 128], f32, name="pa2", space="PSUM")
                nc.tensor.matmul(pa2[:], A2[0:65, isl], W2comb[:, :])
                nc.scalar.copy(apm2[:, i * 128:(i + 1) * 128], pa2[:])
            # v2 = x1 @ W2b -> DRAM rows
            for c in range(4):
                cs = slice(c * 512, (c + 1) * 512)
                pv = ppp.tile([128, 512], f32, name="pv2", space="PSUM")
                nc.tensor.matmul(pv[:], w_c2w1b[:, :], A2[0:64, cs])
                nc.scalar.copy(vscratch[:, cs], pv[:])
            for grp in range(4):
                vstage = pp.tile([128, 512], bf16, name="vstage2")
                for m in range(4):
                    i = grp * 4 + m
                    tvp = ppp.tile([128, 128], f32, name="tvp2", space="PSUM")
                    nc.tensor.transpose(tvp[:], vscratch[:, i * P:(i + 1) * P],
                                        ident_sb[:, :])
                    nc.vector.tensor_copy(vstage[:, m * 128:(m + 1) * 128], tvp[:])
                nc.sync.dma_start(
                    v2d[:, :].rearrange("(g m r) ch -> g r m ch", g=4, m=4)[grp],
                    vstage[:, :])

        # =============== conv2 ===============
        with tc.tile_pool(name="c2", bufs=2) as sp, \
             tc.tile_pool(name="c2ps", bufs=2, space="PSUM") as spp:
            edge_conv(2, sp, spp)

        # =============== classifier ===============
        with tc.tile_pool(name="cls", bufs=2) as cp, \
             tc.tile_pool(name="clsps", bufs=2, space="PSUM") as cpp:
            pooled = g.tile([128, 4], f32, name="pooled")
            for t_ in range(4):
                tsl = slice(t_ * 128, (t_ + 1) * 128)
                ps = cpp.tile([128, 2048], f32, name="ps_l0", tag="ps_l0", bufs=1)
                for c in range(4):
                    cs = slice(c * 512, (c + 1) * 512)
                    nc.tensor.matmul(ps[:, cs], mm(w_l0w[:, 0:512][:, tsl]),
                                     mm(x2Ta[:, cs]), start=True, stop=False)
                    nc.tensor.matmul(ps[:, cs], mm(w_l0w[:, 512:1024][:, tsl]),
                                     mm(x2Tb[:, cs]), start=False, stop=True)
                pool1 = cp.tile([128, 1], f32, name="pool1")
                nc.vector.tensor_reduce(pool1[:, :], ps[:, :], axis=AX.X, op=OP.max)
                nc.scalar.activation(pooled[:, t_:t_ + 1], pool1[:, :],
                                     ACT.Relu, bias=b_l0b[:, t_:t_ + 1])
            # l1: 512 -> 256
            y1 = g.tile([128, 2], f32, name="y1")
            for h in range(2):
                ps1 = cpp.tile([128, 1], f32, name="ps_l1", tag="ps_s")
                for c in range(4):
                    nc.tensor.matmul(ps1[:],
                                     w_l1w[:, c * 256 + h * 128: c * 256 + (h + 1) * 128].bitcast(f32),
                                     pooled[:, c:c + 1],
                                     start=(c == 0), stop=(c == 3))
                nc.scalar.activation(y1[:, h:h + 1], ps1[:, :], ACT.Relu,
                                     bias=b_l1b[:, h:h + 1])
            # l2: 256 -> 256
            y2 = g.tile([128, 2], f32, name="y2")
            for h in range(2):
                ps2_ = cpp.tile([128, 1], f32, name="ps_l2", tag="ps_s")
                for c in range(2):
                    nc.tensor.matmul(ps2_[:],
                                     w_l2w[:, c * 256 + h * 128: c * 256 + (h + 1) * 128].bitcast(f32),
                                     y1[:, c:c + 1],
                                     start=(c == 0), stop=(c == 1))
                nc.scalar.activation(y2[:, h:h + 1], ps2_[:, :], ACT.Relu,
                                     bias=b_l2b[:, h:h + 1])
            # l3: 256 -> 40
            ps3 = cpp.tile([NUM_CLASSES, 1], f32, name="ps_l3", tag="ps_s")
            for c in range(2):
                nc.tensor.matmul(ps3[:],
                                 w_l3w[:, c * NUM_CLASSES:(c + 1) * NUM_CLASSES].bitcast(f32),
                                 y2[:, c:c + 1],
                                 start=(c == 0), stop=(c == 1))
            y3 = cp.tile([NUM_CLASSES, 1], f32, name="y3")
            nc.vector.tensor_add(y3[:, :], ps3[:, :], b_l3b[:, :])
            # log_softmax over the 40 values: transpose to one row
            pr = cpp.tile([1, NUM_CLASSES], f32, name="pr", tag="ps_s")
            nc.tensor.transpose(pr[:], y3[:, :], ident_sb[0:NUM_CLASSES, 0:NUM_CLASSES])
            row = cp.tile([1, NUM_CLASSES], f32, name="row")
            nc.vector.tensor_copy(row[:, :], pr[:, :])
            mx = cp.tile([1, 1], f32, name="mx")
            nc.vector.tensor_reduce(mx[:, :], row[:, :], axis=AX.X, op=OP.max)
            nmx = cp.tile([1, 1], f32, name="nmx")
            nc.scalar.mul(nmx[:, :], mx[:, :], -1.0)
            ex = cp.tile([1, NUM_CLASSES], f32, name="ex")
            sacc = cp.tile([1, 1], f32, name="sacc")
            nc.scalar.activation(ex[:, :], row[:, :], ACT.Exp,
                                 bias=nmx[:, 0:1], accum_out=sacc[:, :])
            lnz = cp.tile([1, 1], f32, name="lnz")
            nc.scalar.activation(lnz[:, :], sacc[:, :], ACT.Ln)
            shift = cp.tile([1, 1], f32, name="shift")
            nc.vector.tensor_sub(shift[:, :], lnz[:, :], nmx[:, :])
            osb = cp.tile([1, NUM_CLASSES], f32, name="osb")
            nc.vector.tensor_scalar(osb[:, :], row[:, :], shift[:, 0:1],
                                    None, op0=OP.subtract)
            nc.sync.dma_start(out[:, :], osb[:, :])

        ctx.close()

    nc.compile()
    return nc


def _get_program():
    if "nc" not in _PROGRAM_CACHE:
        _PROGRAM_CACHE["nc"] = _build_program()
    return _PROGRAM_CACHE["nc"]


def _in_maps(inputs):
    w_names = ["c1w1", "c1b1", "c1w2", "c1b2", "c1w3", "c1b3",
               "c2w1", "c2b1", "c2w2", "c2b2", "c2w3", "c2b3",
               "l0w", "l0b", "l1w", "l1b", "l2w", "l2b", "l3w", "l3b"]
    shared = {k: np.ascontiguousarray(np.asarray(inputs[k], np.float32))
              for k in w_names}
    shared["ident"] = np.eye(128, dtype=np.float32)
    pos = np.ascontiguousarray(np.asarray(inputs["pos"], np.float32))
    maps = []
    for c in range(NCLOUD):
        m = dict(shared)
        m["pos"] = np.ascontiguousarray(pos[c * N:(c + 1) * N])
        maps.append(m)
    return maps


def kernel(**inputs) -> np.ndarray:
    from concourse import bass_utils
    nc = _get_program()
    maps = _in_maps(inputs)
    res = bass_utils.run_bass_kernel_spmd(nc, maps, core_ids=list(range(NCLOUD)))
    outs = [np.asarray(r["out"]).reshape(1, NUM_CLASSES) for r in res.results]
    return np.concatenate(outs, axis=0).astype(np.float32)


# revision 11
# speedup vs baseline: 1.0336x; 1.0198x over previous
"""DGCNN (dynamic edge conv x2 + classifier) Trainium2 Bass kernel.

Sharding: data-parallel over the 8 point clouds -> 8 NeuronCores.
Each core runs the full per-cloud pipeline:
  conv1: kNN in 3-D, edge MLP 6->64->64->64, max over K=20
  conv2: kNN in 64-D feature space, edge MLP 128->128->128->256, max over K
  head : 256->512, global max pool, 512->256->256->40, log_softmax

Key per-core implementation ideas:
  * kNN scores via one augmented matmul: [x,1] @ [-2x; |x|^2]^T.
  * composite sort keys: each u32 word = fp16(-d2) in the high 16 bits
    (written by the scalar engine straight from PSUM with a strided AP)
    and a persistent u16 column-index iota in the low 16 bits.  Read as
    f32, lexicographic float order ranks by distance with deterministic
    index tie-breaks, so DVE max8 alone yields both value and index --
    no max_index scans, and neighbor indices pop out with a bitwise AND.
  * top-24 via 3 rounds of max8; match_replace writes into a scratch
    copy so the key tile (and its iota) is never clobbered.
  * per-round batched indirect-DMA gathers (8 offsets per call) instead
    of one call per neighbor: 3 GPSIMD descriptor-gen ops per tile.
  * edge first layer decomposed: z1 = relu(a_i + v_j) with
    a = x@(W1a-W1b)+b1 (point-major, one small matmul per tile) and
    v = x@W1b gathered from DRAM; the broadcast add runs point-major as
    a single DVE op over all 20 neighbors before the PE transposes.
  * layer-3 outputs for a whole tile land in one multi-bank PSUM tile;
    the max over K collapses to one strided tensor_reduce per half.
"""

import os
import sys
import numpy as np

for _p in ("/opt/trn_rl_repo",):
    if _p not in sys.path:
        sys.path.insert(0, _p)

N = 2048          # points per cloud
NCLOUD = 8
P = 128           # partition tile
NT = N // P       # 16 row tiles
KNN = 20
KSEL = 24         # 3 rounds x 8
NEG_BIG = -3.0e38
NUM_CLASSES = 40

# matmul dtype knobs: None -> plain float32; "f32r" -> float32r fast path
MM_FAST_MLP = True     # edge-MLP layers 2/3 + classifier matmuls
MM_FAST_SCORES = True  # kNN score matmuls: f32r (the fp16 sort keys already
                       # quantize d2 to ~2^-11 relative, so f32r's reduced
                       # multiply precision is in the same noise class)

_PROGRAM_CACHE = {}


def _build_program():
    import concourse.bass as bass
    import concourse.bacc as bacc
    import concourse.tile as tile
    from concourse import mybir

    f32 = mybir.dt.float32
    f32r = mybir.dt.float32r
    f16 = mybir.dt.float16
    bf16 = mybir.dt.bfloat16
    u16 = mybir.dt.uint16
    u32 = mybir.dt.uint32
    AX = mybir.AxisListType
    OP = mybir.AluOpType
    ACT = mybir.ActivationFunctionType

    def mm(ap):
        return ap.bitcast(f32r) if MM_FAST_MLP else ap

    def mms(ap):
        return ap.bitcast(f32r) if MM_FAST_SCORES else ap

    mmo = mm   # producer outputs feeding fast matmuls must round to f32r
    mmso = mms  # same, for the score-matmul operands A*/B*

    nc = bacc.Bacc("TRN2", target_bir_lowering=False, debug=False)

    # ---------------- I/O ----------------
    def din(name, shape):
        return nc.dram_tensor(name, list(shape), f32, kind="ExternalInput").ap()

    pos = din("pos", [N, 3])
    c1w1 = din("c1w1", [6, 64]);   c1b1 = din("c1b1", [64])
    c1w2 = din("c1w2", [64, 64]);  c1b2 = din("c1b2", [64])
    c1w3 = din("c1w3", [64, 64]);  c1b3 = din("c1b3", [64])
    c2w1 = din("c2w1", [128, 128]); c2b1 = din("c2b1", [128])
    c2w2 = din("c2w2", [128, 128]); c2b2 = din("c2b2", [128])
    c2w3 = din("c2w3", [128, 256]); c2b3 = din("c2b3", [256])
    l0w = din("l0w", [256, 512]);  l0b = din("l0b", [512])
    l1w = din("l1w", [512, 256]);  l1b = din("l1b", [256])
    l2w = din("l2w", [256, 256]);  l2b = din("l2b", [256])
    l3w = din("l3w", [256, NUM_CLASSES]); l3b = din("l3b", [NUM_CLASSES])
    ident = din("ident", [128, 128])

    out = nc.dram_tensor("out", [1, NUM_CLASSES], f32, kind="ExternalOutput").ap()

    with tile.TileContext(nc) as tc:
        from contextlib import ExitStack

        ctx = ExitStack()
        g = ctx.enter_context(tc.tile_pool(name="g", bufs=1))          # persistent
        dpool = ctx.enter_context(tc.tile_pool(name="dram", bufs=1, space="DRAM"))

        # persistent SBUF state
        ident_sb = g.tile([128, 128], f32)
        nc.sync.dma_start(ident_sb[:], ident[:, :])
        ident_bf = g.tile([128, 128], bf16)
        nc.scalar.copy(ident_bf[:, :], ident_sb[:, :])

        A1 = g.tile([4, N], f32)       # [x^T ; 1]
        B1 = g.tile([4, N], f32)       # [-2 x^T ; s]
        A2 = g.tile([65, N], f32)      # [x1^T ; 1]
        B2 = g.tile([65, N], f32)      # [-2 x1^T ; s2]
        x2Ta = g.tile([128, N], f32)   # conv2 out ch 0:128
        x2Tb = g.tile([128, N], f32)   # conv2 out ch 128:256
        nscol1 = g.tile([128, NT], f32)  # -s_i per tile column
        nscol2 = g.tile([128, NT], f32)
        vscratch = g.tile([128, N], f32, name="vscratch")  # v1T/x1sq/v2T staging
        apm1 = g.tile([128, NT * 64], f32, name="apm1")    # a_i point-major
        apm2 = g.tile([128, NT * 128], bf16, name="apm2")

        # composite-key ring: u32 word = fp16(-d2) << 16 | column index.
        # The u16 iota in the low halves persists across tiles and convs;
        # only the fp16 halves are rewritten (strided ACT store from PSUM).
        KB = 3
        keybufs = [g.tile([128, 2 * N], u16, name=f"key{r}") for r in range(KB)]
        for kb in keybufs:
            ev = kb.rearrange("p (n two) -> p n two", two=2)[:, :, 0:1]
            nc.gpsimd.iota(ev, pattern=[[1, N]], base=0, channel_multiplier=0)

        v1d = dpool.tile([N, 64], f32, name="v1d")
        v2d = dpool.tile([N, 128], bf16, name="v2d")

        # weights / biases.  Weights consumed by f32r matmuls are loaded into
        # a scratch tile and rounded into an f32r-typed tile with an ACT copy
        # (the BIR verifier requires every writer of an f32r matmul operand
        # to emit rounded data, so the DMA cannot write them directly).
        wraw = ctx.enter_context(tc.tile_pool(name="wraw", bufs=2))

        def load_w(name, shape, pieces, wdt=None):
            wdt = wdt or (f32r if MM_FAST_MLP else f32)
            if wdt != f32:
                raw = wraw.tile(list(shape), f32, name=name + "_raw", tag="wraw")
                for sl, srcap in pieces:
                    nc.sync.dma_start(raw[sl], srcap)
                t = g.tile(list(shape), wdt, name=name)
                nc.scalar.copy(t[:, :], raw[:, :])
            else:
                t = g.tile(list(shape), f32, name=name)
                for sl, srcap in pieces:
                    nc.sync.dma_start(t[sl], srcap)
            return t

        SALL = (slice(None), slice(None))
        w_c1w1a = g.tile([3, 64], f32); nc.sync.dma_start(w_c1w1a[:], c1w1[0:3, :])
        w_c1w1b = g.tile([3, 64], f32); nc.sync.dma_start(w_c1w1b[:], c1w1[3:6, :])
        w_c2w1a = g.tile([64, 128], f32); nc.sync.dma_start(w_c2w1a[:], c2w1[0:64, :])
        w_c2w1b = g.tile([64, 128], f32); nc.sync.dma_start(w_c2w1b[:], c2w1[64:128, :])
        w_c1w2 = load_w("w_c1w2", [64, 64], [(SALL, c1w2[:, :])])
        w_c1w3 = load_w("w_c1w3", [64, 64], [(SALL, c1w3[:, :])])
        w_c2w2 = load_w("w_c2w2", [128, 128], [(SALL, c2w2[:, :])], wdt=bf16)
        w_c2w3 = load_w("w_c2w3", [128, 256], [(SALL, c2w3[:, :])], wdt=bf16)
        w_l0w = load_w("w_l0w", [128, 1024],
                       [((slice(None), slice(0, 512)), l0w[0:128, :]),
                        ((slice(None), slice(512, 1024)), l0w[128:256, :])])
        w_l1w = load_w("w_l1w", [128, 1024],
                       [((slice(None), slice(c * 256, (c + 1) * 256)),
                         l1w[c * 128:(c + 1) * 128, :]) for c in range(4)])
        w_l2w = load_w("w_l2w", [128, 512],
                       [((slice(None), slice(0, 256)), l2w[0:128, :]),
                        ((slice(None), slice(256, 512)), l2w[128:256, :])])
        w_l3w = load_w("w_l3w", [128, 2 * NUM_CLASSES],
                       [((slice(None), slice(0, NUM_CLASSES)), l3w[0:128, :]),
                        ((slice(None), slice(NUM_CLASSES, 2 * NUM_CLASSES)), l3w[128:256, :])])

        # first-layer combined weights: a = x @ (W1a - W1b) + b  (point-major)
        W1comb = g.tile([4, 64], f32, name="W1comb")
        nc.vector.tensor_sub(W1comb[0:3, :], w_c1w1a[:, :], w_c1w1b[:, :])
        nc.sync.dma_start(W1comb[3:4, :], c1b1.rearrange("(o c) -> o c", o=1))
        W2comb = g.tile([65, 128], f32, name="W2comb")
        nc.vector.tensor_sub(W2comb[0:64, :], w_c2w1a[:, :], w_c2w1b[:, :])
        nc.sync.dma_start(W2comb[64:65, :], c2b1.rearrange("(o c) -> o c", o=1))

        def col(name, src, n):
            t = g.tile([n, 1], f32, name=name)
            nc.sync.dma_start(t[:, :], src.rearrange("(c o) -> c o", o=1))
            return t

        b_c1b2 = col("b_c1b2", c1b2, 64)
        b_c1b3 = col("b_c1b3", c1b3, 64)
        b_c2b2 = col("b_c2b2", c2b2, 128)
        b_c2b3 = g.tile([128, 2], f32)
        nc.sync.dma_start(b_c2b3[:, 0:1], c2b3.rearrange("(h c o) -> h c o", h=2, o=1)[0])
        nc.sync.dma_start(b_c2b3[:, 1:2], c2b3.rearrange("(h c o) -> h c o", h=2, o=1)[1])
        b_l0b = g.tile([128, 4], f32)
        for t_ in range(4):
            nc.sync.dma_start(b_l0b[:, t_:t_ + 1],
                              l0b.rearrange("(h c o) -> h c o", h=4, o=1)[t_])
        b_l1b = g.tile([128, 2], f32)
        for t_ in range(2):
            nc.sync.dma_start(b_l1b[:, t_:t_ + 1],
                              l1b.rearrange("(h c o) -> h c o", h=2, o=1)[t_])
        b_l2b = g.tile([128, 2], f32)
        for t_ in range(2):
            nc.sync.dma_start(b_l2b[:, t_:t_ + 1],
                              l2b.rearrange("(h c o) -> h c o", h=2, o=1)[t_])
        b_l3b = col("b_l3b", l3b, NUM_CLASSES)

        # engines cannot address partition bases 3/64 directly: stage a ones
        # row at partition 0 and DMA it into place
        ones_row = g.tile([1, N], f32, name="ones_row")
        nc.vector.memset(ones_row[:, :], 1.0)
        nc.sync.dma_start(A1[3:4, :], ones_row[:, :])
        nc.sync.dma_start(A2[64:65, :], ones_row[:, :])
        # (re-rounded to f32r in conv2 prep, after conv1 has run)

        # =============== conv1 prep ===============
        with tc.tile_pool(name="prep", bufs=2) as pp, \
             tc.tile_pool(name="prep_ps", bufs=2, space="PSUM") as ppp:
            scol = g.tile([128, NT], f32, name="scol1_pos")
            for i in range(NT):
                isl = slice(i * P, (i + 1) * P)
                pt = pp.tile([128, 3], f32, name="pt")
                nc.sync.dma_start(pt[:], pos[isl, :])
                sq = pp.tile([128, 3], f32, name="sq")
                nc.scalar.activation(sq[:], pt[:], ACT.Square,
                                     accum_out=scol[:, i:i + 1])
                tp = ppp.tile([3, 128], f32, name="tp", space="PSUM", bufs=1)
                nc.tensor.transpose(tp[:], pt[:], ident_sb[:])
                nc.scalar.copy(mmso(A1[0:3, isl]), tp[:])
            nc.scalar.mul(nscol1[:, :], scol[:, :], -1.0)
            nc.scalar.mul(mmso(B1[0:3, :]), A1[0:3, :], -2.0)
            # s row: transpose scol [128, NT] -> [NT, 128], stage in SBUF, then
            # one cross-partition DMA into B1 row 3 (engines cannot shift
            # partitions; DMA can).
            stp = ppp.tile([NT, 128], f32, name="stp", space="PSUM", bufs=1)
            nc.tensor.transpose(stp[:], scol[:, :], ident_sb[:])
            srow_sb = pp.tile([NT, 128], f32, name="srow_sb")
            nc.scalar.copy(srow_sb[:, :], stp[:, :])
            nc.sync.dma_start(
                B1[3:4, :].rearrange("o (p n) -> o p n", p=NT), srow_sb[:, :])

            # a_i point-major: one small matmul per tile (A1 rows carry [x;1])
            for i in range(NT):
                isl = slice(i * P, (i + 1) * P)
                pa = ppp.tile([128, 64], f32, name="pa", space="PSUM")
                nc.tensor.matmul(pa[:], A1[0:4, isl], W1comb[:, :])
                nc.scalar.copy(apm1[:, i * 64:(i + 1) * 64], pa[:])

            # v1 = x @ W1b, channel-major; rows -> DRAM [N, 64]
            for c in range(4):
                cs = slice(c * 512, (c + 1) * 512)
                pv = ppp.tile([64, 512], f32, name="pv", space="PSUM")
                nc.tensor.matmul(pv[:], w_c1w1b[:, :], A1[0:3, cs])
                nc.scalar.copy(vscratch[0:64, cs], pv[:])
            for grp in range(4):
                vstage = pp.tile([128, 256], f32, name="vstage")
                for m in range(4):
                    i = grp * 4 + m
                    tvp = ppp.tile([128, 64], f32, name="tvp", space="PSUM")
                    nc.tensor.transpose(tvp[:], vscratch[0:64, i * P:(i + 1) * P],
                                        ident_sb[0:64, 0:64])
                    nc.vector.tensor_copy(vstage[:, m * 64:(m + 1) * 64], tvp[:])
                nc.sync.dma_start(
                    v1d[:, :].rearrange("(g m r) ch -> g r m ch", g=4, m=4)[grp],
                    vstage[:, :])

        # =============== edge-conv block (shared structure) ===============
        def edge_conv(conv, sp, spp):
            """conv=1: H=64 channels; conv=2: H=128 (256 out)."""
            if conv == 1:
                H, CON, WD, ED, idt = 64, 4, 64, f32, ident_sb
                Asb, Bsb, vd, nscol, apm = A1, B1, v1d, nscol1, apm1
                wl2, wl3 = w_c1w2, w_c1w3
                bl2, bl3 = b_c1b2, b_c1b3
                W = mm        # f32r bitcast for conv1 MLP
            else:
                H, CON, WD, ED, idt = 128, 65, 128, bf16, ident_bf
                Asb, Bsb, vd, nscol, apm = A2, B2, v2d, nscol2, apm2
                wl2, wl3 = w_c2w2, w_c2w3
                bl2, bl3 = b_c2b2, b_c2b3
                W = lambda ap: ap   # tiles already bf16
            nhalf = 1 if conv == 1 else 2

            state = {}

            def stage_scores(i):
                isl = slice(i * P, (i + 1) * P)
                key = keybufs[i % KB]
                keyh = key.bitcast(f16).rearrange("p (n two) -> p n two", two=2)
                for c in range(4):
                    cs = slice(c * 512, (c + 1) * 512)
                    psc = spp.tile([128, 512], f32, name="psc", tag="psc", bufs=2)
                    nc.tensor.matmul(psc[:, :],
                                     mms(Asb[0:CON, isl]), mms(Bsb[0:CON, cs]))
                    # fp16(-d2) into the high u16 halves (strided store)
                    nc.scalar.activation(
                        keyh[:, cs, 1:2],
                        psc[:, :].rearrange("p (n o) -> p n o", o=1),
                        ACT.Identity, bias=nscol[:, i:i + 1], scale=-1.0)
                state[i] = {"key": key}

            def stage_topk(i):
                # 3 rounds of max8 over the composite keys; match_replace
                # writes into a scratch copy so the key iota survives.  The
                # neighbor index is the low 16 bits of each winning key.
                key32 = state[i]["key"].bitcast(f32)
                vals = sp.tile([128, KSEL], u32, name="vals", tag="vals", bufs=4)
                valsf = vals.bitcast(f32)
                idx = sp.tile([128, KSEL], u32, name="idx", tag="idx", bufs=4)
                scr = sp.tile([128, N], f32, name="scr", tag="scr", bufs=2)
                for r in range(3):
                    rs = slice(r * 8, (r + 1) * 8)
                    src = key32 if r == 0 else scr[:, :]
                    nc.vector.max(valsf[:, rs], src)
                    if r < 2:
                        nc.vector.match_replace(scr[:, :], valsf[:, rs], src,
                                                NEG_BIG)
                    nc.vector.tensor_scalar(idx[:, rs], vals[:, rs],
                                            0x7FF, None, op0=OP.bitwise_and)
                state[i].update(idx=idx)

            def stage_gather(i):
                # HW indirect DMA consumes ONE offset per destination
                # partition, so issue one gather per neighbor slot.  Runs a
                # full pipeline stage ahead of the consumer so the ~1.1us
                # per-call GPSIMD descriptor-gen cost overlaps tile i-1's
                # MLP instead of stalling it.
                idx = state[i]["idx"]
                gath = sp.tile([128, KNN * WD], ED, name="gath", tag="gath", bufs=3)
                for k in range(KNN):
                    nc.gpsimd.indirect_dma_start(
                        out=gath[:, k * WD:(k + 1) * WD], out_offset=None,
                        in_=vd[:, :],
                        in_offset=bass.IndirectOffsetOnAxis(
                            ap=idx[:, k:k + 1], axis=0),
                    )
                state[i].update(gath=gath)

            def stage_mlp_a(i):
                # z1 = relu(a_i + v_j) point-major, then PE transposes to
                # channel-major; L2; L3 matmuls for half 0.
                isl = slice(i * P, (i + 1) * P)
                gath = state[i]["gath"]
                gv = gath.rearrange("p (k c) -> p k c", k=KNN)
                av = apm[:, i * WD:(i + 1) * WD] \
                    .rearrange("p (o c) -> p o c", o=1) \
                    .to_broadcast([128, KNN, WD])
                nc.vector.tensor_add(gv, gv, av)
                nc.scalar.activation(gath[:, :], gath[:, :], ACT.Relu)

                z1T = sp.tile([H, KNN * 128], ED, name="z1T", tag="z1T", bufs=2)
                for c in range(5):
                    cs = slice(c * 512, (c + 1) * 512)
                    ptr = spp.tile([H, 512], ED, name="ptr", tag="ptr", bufs=1,
                                   space="PSUM")
                    for m in range(4):
                        k = c * 4 + m
                        nc.tensor.transpose(
                            ptr[:, m * 128:(m + 1) * 128],
                            gath[:, k * WD:(k + 1) * WD],
                            idt[:, :])
                    nc.scalar.copy(W(z1T[:, cs]), ptr[:, :])
                # ---- layer 2 ----
                z2T = sp.tile([H, KNN * 128], ED, name="z2T", tag="z2T", bufs=2)
                for c in range(5):
                    cs = slice(c * 512, (c + 1) * 512)
                    pm = spp.tile([H, 512], f32, name="pm", tag="pm", bufs=1,
                                  space="PSUM")
                    nc.tensor.matmul(pm[:], W(wl2[:, :]), W(z1T[:, cs]))
                    nc.scalar.activation(W(z2T[:, cs]), pm[:], ACT.Relu,
                                         bias=bl2[:, 0:1])
                # ---- layer 3, half 0: chunks 0-3 into one 4-bank PSUM
                # tile; chunk 4 borrows the pm-tag bank (L2 is done with it)
                wsel = wl3[:, :] if conv == 1 else wl3[:, 0:128]
                pl = spp.tile([H, 4 * 512], f32, name="pl", tag="pl", bufs=1,
                              space="PSUM")
                for c in range(4):
                    nc.tensor.matmul(pl[:, c * 512:(c + 1) * 512],
                                     W(wsel), W(z2T[:, c * 512:(c + 1) * 512]))
                pl4 = spp.tile([H, 512], f32, name="pl4", tag="pm", bufs=1,
                               space="PSUM")
                nc.tensor.matmul(pl4[:], W(wsel), W(z2T[:, 4 * 512:5 * 512]))
                state[i].update(z2T=z2T, pl=pl, pl4=pl4)

            def stage_mlp_b(i):
                # max over K for half 0 (one strided reduce over 5 banks),
                # then L3 matmuls for half 1 (conv2) reusing the same banks.
                isl = slice(i * P, (i + 1) * P)
                pl, pl4 = state[i]["pl"], state[i]["pl4"]
                redf = sp.tile([128, 128], f32, name="redf", tag="redf", bufs=2)
                red4 = sp.tile([128, 128], f32, name="red4", tag="red4", bufs=2)
                nc.vector.tensor_reduce(
                    redf[0:H, :],
                    pl[:, :].rearrange("p (c k n) -> p n (c k)", c=4, k=4),
                    axis=AX.X, op=OP.max)
                nc.vector.tensor_reduce(
                    red4[0:H, :],
                    pl4[:, :].rearrange("p (k n) -> p n k", k=4),
                    axis=AX.X, op=OP.max)
                nc.vector.tensor_tensor(redf[0:H, :], redf[0:H, :],
                                        red4[0:H, :], op=OP.max)
                if conv == 1:
                    nc.scalar.activation(mmso(A2[0:64, isl]), redf[0:64, :],
                                         ACT.Relu, bias=bl3[:, 0:1])
                else:
                    nc.scalar.activation(mmo(x2Ta[:, isl]), redf[:, :],
                                         ACT.Relu, bias=bl3[:, 0:1])
                    z2T = state[i]["z2T"]
                    pl2 = spp.tile([H, 4 * 512], f32, name="pl2", tag="pl",
                                   bufs=1, space="PSUM")
                    for c in range(4):
                        nc.tensor.matmul(
                            pl2[:, c * 512:(c + 1) * 512],
                            wl3[:, 128:256], W(z2T[:, c * 512:(c + 1) * 512]))
                    pl42 = spp.tile([H, 512], f32, name="pl42", tag="pm",
                                    bufs=1, space="PSUM")
                    nc.tensor.matmul(pl42[:], wl3[:, 128:256],
                                     W(z2T[:, 4 * 512:5 * 512]))
                    state[i]["pl2"] = pl2
                    state[i]["pl42"] = pl42

            def stage_mlp_c(i):
                if conv == 1:
                    del state[i]
                    return
                isl = slice(i * P, (i + 1) * P)
                pl2, pl42 = state[i]["pl2"], state[i]["pl42"]
                redf = sp.tile([128, 128], f32, name="redf", tag="redf", bufs=2)
                red4 = sp.tile([128, 128], f32, name="red4", tag="red4", bufs=2)
                nc.vector.tensor_reduce(
                    redf[:, :],
                    pl2[:, :].rearrange("p (c k n) -> p n (c k)", c=4, k=4),
                    axis=AX.X, op=OP.max)
                nc.vector.tensor_reduce(
                    red4[:, :],
                    pl42[:, :].rearrange("p (k n) -> p n k", k=4),
                    axis=AX.X, op=OP.max)
                nc.vector.tensor_tensor(redf[:, :], redf[:, :],
                                        red4[:, :], op=OP.max)
                nc.scalar.activation(mmo(x2Tb[:, isl]), redf[:, :],
                                     ACT.Relu, bias=bl3[:, 1:2])
                del state[i]

            # 3-deep software pipeline: while tile i's MLP runs, the GPSIMD
            # queue drains tile i+1's 20 gathers (issued a full stage after
            # their top-k) and the DVE runs tile i+2's top-k rounds, so no
            # engine waits on same-tile producers.
            stage_scores(0)
            stage_topk(0)
            stage_gather(0)
            if NT > 1:
                stage_scores(1)
                stage_topk(1)
            for i in range(NT):
                stage_mlp_a(i)
                if i + 2 < NT:
                    stage_scores(i + 2)
                if i + 1 < NT:
                    stage_gather(i + 1)
                stage_mlp_b(i)
                if i + 2 < NT:
                    stage_topk(i + 2)
                stage_mlp_c(i)

        # =============== conv1 ===============
        with tc.tile_pool(name="c1", bufs=2) as sp, \
             tc.tile_pool(name="c1ps", bufs=2, space="PSUM") as spp:
            edge_conv(1, sp, spp)

        # =============== conv2 prep ===============
        with tc.tile_pool(name="prep2", bufs=2) as pp, \
             tc.tile_pool(name="prep2_ps", bufs=2, space="PSUM") as ppp:
            # s2 row + s2 columns (engines cannot shift partitions: stage the
            # row at partition 0, DMA it into B2 row 64)
            nc.scalar.activation(vscratch[0:64, :], A2[0:64, :], ACT.Square)
            ones64 = g.tile([64, 1], f32, name="ones64")
            nc.vector.memset(ones64[:, :], 1.0)
            s2tmp = pp.tile([1, N], f32, name="s2tmp")
            for c in range(4):
                cs = slice(c * 512, (c + 1) * 512)
                ps2 = ppp.tile([1, 512], f32, name="ps2", space="PSUM", bufs=1)
                nc.tensor.matmul(ps2[:], ones64[:, :], vscratch[0:64, cs])
                nc.scalar.copy(s2tmp[0:1, cs], ps2[:])
            nc.sync.dma_start(B2[64:65, :], s2tmp[:, :])
            for i in range(NT):
                isl = slice(i * P, (i + 1) * P)
                tsc = ppp.tile([128, 1], f32, name="tsc", space="PSUM", bufs=1)
                nc.tensor.transpose(tsc[:], s2tmp[0:1, isl], ident_sb[0:1, 0:1])
                nc.scalar.mul(nscol2[:, i:i + 1], tsc[:], -1.0)
            nc.scalar.mul(mmso(B2[0:64, :]), A2[0:64, :], -2.0)
            # a_i point-major (A2 rows carry [x1;1], W2comb = [W2a-W2b; b2])
            for i in range(NT):
                isl = slice(i * P, (i + 1) * P)
                pa2 = ppp.tile([128, 128], f32, name="pa2", space="PSUM")
                nc.tensor.matmul(pa2[:], A2[0:65, isl], W2comb[:, :])
                nc.scalar.copy(apm2[:, i * 128:(i + 1) * 128], pa2[:])
            # v2 = x1 @ W2b -> DRAM rows
            for c in range(4):
                cs = slice(c * 512, (c + 1) * 512)
                pv = ppp.tile([128, 512], f32, name="pv2", space="PSUM")
                nc.tensor.matmul(pv[:], w_c2w1b[:, :], A2[0:64, cs])
                nc.scalar.copy(vscratch[:, cs], pv[:])
            for grp in range(4):
                vstage = pp.tile([128, 512], bf16, name="vstage2")
                for m in range(4):
                    i = grp * 4 + m
                    tvp = ppp.tile([128, 128], f32, name="tvp2", space="PSUM")
                    nc.tensor.transpose(tvp[:], vscratch[:, i * P:(i + 1) * P],
                                        ident_sb[:, :])
                    nc.vector.tensor_copy(vstage[:, m * 128:(m + 1) * 128], tvp[:])
                nc.sync.dma_start(
                    v2d[:, :].rearrange("(g m r) ch -> g r m ch", g=4, m=4)[grp],
                    vstage[:, :])

        # =============== conv2 ===============
        with tc.tile_pool(name="c2", bufs=2) as sp, \
             tc.tile_pool(name="c2ps", bufs=2, space="PSUM") as spp:
            edge_conv(2, sp, spp)

        # =============== classifier ===============
        with tc.tile_pool(name="cls", bufs=2) as cp, \
             tc.tile_pool(name="clsps", bufs=2, space="PSUM") as cpp:
            pooled = g.tile([128, 4], f32, name="pooled")
            for t_ in range(4):
                tsl = slice(t_ * 128, (t_ + 1) * 128)
                ps = cpp.tile([128, 2048], f32, name="ps_l0", tag="ps_l0", bufs=1)
                for c in range(4):
                    cs = slice(c * 512, (c + 1) * 512)
                    nc.tensor.matmul(ps[:, cs], mm(w_l0w[:, 0:512][:, tsl]),
                                     mm(x2Ta[:, cs]), start=True, stop=False)
                    nc.tensor.matmul(ps[:, cs], mm(w_l0w[:, 512:1024][:, tsl]),
                                     mm(x2Tb[:, cs]), start=False, stop=True)
                pool1 = cp.tile([128, 1], f32, name="pool1")
                nc.vector.tensor_reduce(pool1[:, :], ps[:, :], axis=AX.X, op=OP.max)
                nc.scalar.activation(pooled[:, t_:t_ + 1], pool1[:, :],
                                     ACT.Relu, bias=b_l0b[:, t_:t_ + 1])
            # l1: 512 -> 256
            y1 = g.tile([128, 2], f32, name="y1")
            for h in range(2):
                ps1 = cpp.tile([128, 1], f32, name="ps_l1", tag="ps_s")
                for c in range(4):
                    nc.tensor.matmul(ps1[:],
                                     w_l1w[:, c * 256 + h * 128: c * 256 + (h + 1) * 128].bitcast(f32),
                                     pooled[:, c:c + 1],
                                     start=(c == 0), stop=(c == 3))
                nc.scalar.activation(y1[:, h:h + 1], ps1[:, :], ACT.Relu,
                                     bias=b_l1b[:, h:h + 1])
            # l2: 256 -> 256
            y2 = g.tile([128, 2], f32, name="y2")
            for h in range(2):
                ps2_ = cpp.tile([128, 1], f32, name="ps_l2", tag="ps_s")
                for c in range(2):
                    nc.tensor.matmul(ps2_[:],
                                     w_l2w[:, c * 256 + h * 128: c * 256 + (h + 1) * 128].bitcast(f32),
                                     y1[:, c:c + 1],
                                     start=(c == 0), stop=(c == 1))
                nc.scalar.activation(y2[:, h:h + 1], ps2_[:, :], ACT.Relu,
                                     bias=b_l2b[:, h:h + 1])
            # l3: 256 -> 40
            ps3 = cpp.tile([NUM_CLASSES, 1], f32, name="ps_l3", tag="ps_s")
            for c in range(2):
                nc.tensor.matmul(ps3[:],
                                 w_l3w[:, c * NUM_CLASSES:(c + 1) * NUM_CLASSES].bitcast(f32),
                                 y2[:, c:c + 1],
                                 start=(c == 0), stop=(c == 1))
            y3 = cp.tile([NUM_CLASSES, 1], f32, name="y3")
            nc.vector.tensor_add(y3[:, :], ps3[:, :], b_l3b[:, :])
            # log_softmax over the 40 values: transpose to one row
            pr = cpp.tile([1, NUM_CLASSES], f32, name="pr", tag="ps_s")
            nc.tensor.transpose(pr[:], y3[:, :], ident_sb[0:NUM_CLASSES, 0:NUM_CLASSES])
            row = cp.tile([1, NUM_CLASSES], f32, name="row")
            nc.vector.tensor_copy(row[:, :], pr[:, :])
            mx = cp.tile([1, 1], f32, name="mx")
            nc.vector.tensor_reduce(mx[:, :], row[:, :], axis=AX.X, op=OP.max)
            nmx = cp.tile([1, 1], f32, name="nmx")
            nc.scalar.mul(nmx[:, :], mx[:, :], -1.0)
            ex = cp.tile([1, NUM_CLASSES], f32, name="ex")
            sacc = cp.tile([1, 1], f32, name="sacc")
            nc.scalar.activation(ex[:, :], row[:, :], ACT.Exp,
                                 bias=nmx[:, 0:1], accum_out=sacc[:, :])
            lnz = cp.tile([1, 1], f32, name="lnz")
            nc.scalar.activation(lnz[:, :], sacc[:, :], ACT.Ln)
            shift = cp.tile([1, 1], f32, name="shift")
            nc.vector.tensor_sub(shift[:, :], lnz[:, :], nmx[:, :])
            osb = cp.tile([1, NUM_CLASSES], f32, name="osb")
            nc.vector.tensor_scalar(osb[:, :], row[:, :], shift[:, 0:1],
                                    None, op0=OP.subtract)
            nc.sync.dma_start(out[:, :], osb[:, :])

        ctx.close()

    nc.compile()
    return nc


def _get_program():
    if "nc" not in _PROGRAM_CACHE:
        _PROGRAM_CACHE["nc"] = _build_program()
    return _PROGRAM_CACHE["nc"]


def _in_maps(inputs):
    w_names = ["c1w1", "c1b1", "c1w2", "c1b2", "c1w3", "c1b3",
               "c2w1", "c2b1", "c2w2", "c2b2", "c2w3", "c2b3",
               "l0w", "l0b", "l1w", "l1b", "l2w", "l2b", "l3w", "l3b"]
    shared = {k: np.ascontiguousarray(np.asarray(inputs[k], np.float32))
              for k in w_names}
    shared["ident"] = np.eye(128, dtype=np.float32)
    pos = np.ascontiguousarray(np.asarray(inputs["pos"], np.float32))
    maps = []
    for c in range(NCLOUD):
        m = dict(shared)
        m["pos"] = np.ascontiguousarray(pos[c * N:(c + 1) * N])
        maps.append(m)
    return maps


def kernel(**inputs) -> np.ndarray:
    from concourse import bass_utils
    nc = _get_program()
    maps = _in_maps(inputs)
    res = bass_utils.run_bass_kernel_spmd(nc, maps, core_ids=list(range(NCLOUD)))
    outs = [np.asarray(r["out"]).reshape(1, NUM_CLASSES) for r in res.results]
    return np.concatenate(outs, axis=0).astype(np.float32)


# revision 14
# speedup vs baseline: 1.0757x; 1.0407x over previous
"""DGCNN (dynamic edge conv x2 + classifier) Trainium2 Bass kernel.

Sharding: data-parallel over the 8 point clouds -> 8 NeuronCores.
Each core runs the full per-cloud pipeline:
  conv1: kNN in 3-D, edge MLP 6->64->64->64, max over K=20
  conv2: kNN in 64-D feature space, edge MLP 128->128->128->256, max over K
  head : 256->512, global max pool, 512->256->256->40, log_softmax

Key per-core implementation ideas:
  * kNN scores via one augmented matmul: [x,1] @ [-2x; |x|^2]^T.
  * composite sort keys: each u32 word = fp16(-d2) in the high 16 bits
    (written by the scalar engine straight from PSUM with a strided AP)
    and a persistent u16 column-index iota in the low 16 bits.  Read as
    f32, lexicographic float order ranks by distance with deterministic
    index tie-breaks, so DVE max8 alone yields both value and index --
    no max_index scans, and neighbor indices pop out with a bitwise AND.
  * top-24 via 3 rounds of max8; match_replace writes into a scratch
    copy so the key tile (and its iota) is never clobbered.
  * gathers run one full pipeline stage ahead of their consumer (3-deep
    software pipeline: mlp(i) | gathers(i+1) | top-k(i+2)) so the ~1.1us
    per-call GPSIMD descriptor-generation cost overlaps the MLP.
  * score matmuls in float32r (4x PE throughput); the fp16 keys already
    quantize d2 to ~2^-11 relative so the reduced multiply precision is
    in the same noise class.
  * edge first layer decomposed: z1 = relu(a_i + v_j) with
    a = x@(W1a-W1b)+b1 (point-major, one small matmul per tile) and
    v = x@W1b gathered from DRAM; the broadcast add runs point-major as
    a single DVE op over all 20 neighbors before the PE transposes.
  * layer-3 outputs for a whole tile land in one multi-bank PSUM tile;
    the max over K collapses to one strided tensor_reduce per half.
"""

import os
import sys
import numpy as np

for _p in ("/opt/trn_rl_repo",):
    if _p not in sys.path:
        sys.path.insert(0, _p)

N = 2048          # points per cloud
NCLOUD = 8
P = 128           # partition tile
NT = N // P       # 16 row tiles
KNN = 20
KSEL = 24         # 3 rounds x 8
NEG_BIG = -3.0e38
NUM_CLASSES = 40

# matmul dtype knobs: None -> plain float32; "f32r" -> float32r fast path
MM_FAST_MLP = True     # edge-MLP layers 2/3 + classifier matmuls
MM_FAST_SCORES = True  # kNN score matmuls: f32r (the fp16 sort keys already
                       # quantize d2 to ~2^-11 relative, so f32r's reduced
                       # multiply precision is in the same noise class)

_PROGRAM_CACHE = {}


def _build_program():
    import concourse.bass as bass
    import concourse.bacc as bacc
    import concourse.tile as tile
    from concourse import mybir

    f32 = mybir.dt.float32
    f32r = mybir.dt.float32r
    f16 = mybir.dt.float16
    bf16 = mybir.dt.bfloat16
    u16 = mybir.dt.uint16
    u32 = mybir.dt.uint32
    AX = mybir.AxisListType
    OP = mybir.AluOpType
    ACT = mybir.ActivationFunctionType

    def mm(ap):
        return ap.bitcast(f32r) if MM_FAST_MLP else ap

    def mms(ap):
        return ap.bitcast(f32r) if MM_FAST_SCORES else ap

    mmo = mm   # producer outputs feeding fast matmuls must round to f32r
    mmso = mms  # same, for the score-matmul operands A*/B*

    nc = bacc.Bacc("TRN2", target_bir_lowering=False, debug=False,
                   dynamic_dma_scratch_size=49152)

    # ---------------- I/O ----------------
    def din(name, shape):
        return nc.dram_tensor(name, list(shape), f32, kind="ExternalInput").ap()

    pos = din("pos", [N, 3])
    c1w1 = din("c1w1", [6, 64]);   c1b1 = din("c1b1", [64])
    c1w2 = din("c1w2", [64, 64]);  c1b2 = din("c1b2", [64])
    c1w3 = din("c1w3", [64, 64]);  c1b3 = din("c1b3", [64])
    c2w1 = din("c2w1", [128, 128]); c2b1 = din("c2b1", [128])
    c2w2 = din("c2w2", [128, 128]); c2b2 = din("c2b2", [128])
    c2w3 = din("c2w3", [128, 256]); c2b3 = din("c2b3", [256])
    l0w = din("l0w", [256, 512]);  l0b = din("l0b", [512])
    l1w = din("l1w", [512, 256]);  l1b = din("l1b", [256])
    l2w = din("l2w", [256, 256]);  l2b = din("l2b", [256])
    l3w = din("l3w", [256, NUM_CLASSES]); l3b = din("l3b", [NUM_CLASSES])
    ident = din("ident", [128, 128])

    out = nc.dram_tensor("out", [1, NUM_CLASSES], f32, kind="ExternalOutput").ap()

    with tile.TileContext(nc) as tc:
        from contextlib import ExitStack

        ctx = ExitStack()
        g = ctx.enter_context(tc.tile_pool(name="g", bufs=1))          # persistent
        dpool = ctx.enter_context(tc.tile_pool(name="dram", bufs=1, space="DRAM"))

        # persistent SBUF state
        ident_sb = g.tile([128, 128], f32)
        nc.sync.dma_start(ident_sb[:], ident[:, :])
        ident_bf = g.tile([128, 128], bf16)
        nc.scalar.copy(ident_bf[:, :], ident_sb[:, :])

        A1 = g.tile([4, N], f32)       # [x^T ; 1]
        B1 = g.tile([4, N], f32)       # [-2 x^T ; s]
        A2 = g.tile([65, N], f32)      # [x1^T ; 1]
        B2 = g.tile([65, N], f32)      # [-2 x1^T ; s2]
        x2Ta = g.tile([128, N], f32)   # conv2 out ch 0:128
        x2Tb = g.tile([128, N], f32)   # conv2 out ch 128:256
        nscol1 = g.tile([128, NT], f32)  # -s_i per tile column
        nscol2 = g.tile([128, NT], f32)
        vscratch = g.tile([128, N], f32, name="vscratch")  # v1T/x1sq/v2T staging
        apm1 = g.tile([128, NT * 64], f32, name="apm1")    # a_i point-major
        apm2 = g.tile([128, NT * 128], bf16, name="apm2")

        # composite-key ring: u32 word = fp16(-d2) << 16 | column index.
        # The u16 iota in the low halves persists across tiles and convs;
        # only the fp16 halves are rewritten (strided ACT store from PSUM).
        KB = 2
        keybufs = [g.tile([128, 2 * N], u16, name=f"key{r}") for r in range(KB)]
        for kb in keybufs:
            ev = kb.rearrange("p (n two) -> p n two", two=2)[:, :, 0:1]
            nc.gpsimd.iota(ev, pattern=[[1, N]], base=0, channel_multiplier=0)

        v1d = dpool.tile([N, 64], f32, name="v1d")
        v2d = dpool.tile([N, 128], bf16, name="v2d")

        # weights / biases.  Weights consumed by f32r matmuls are loaded into
        # a scratch tile and rounded into an f32r-typed tile with an ACT copy
        # (the BIR verifier requires every writer of an f32r matmul operand
        # to emit rounded data, so the DMA cannot write them directly).
        wraw_ctx = ExitStack()
        wraw = wraw_ctx.enter_context(tc.tile_pool(name="wraw", bufs=2))

        def load_w(name, shape, pieces, wdt=None):
            wdt = wdt or (f32r if MM_FAST_MLP else f32)
            if wdt != f32:
                raw = wraw.tile(list(shape), f32, name=name + "_raw", tag="wraw")
                for sl, srcap in pieces:
                    nc.sync.dma_start(raw[sl], srcap)
                t = g.tile(list(shape), wdt, name=name)
                nc.scalar.copy(t[:, :], raw[:, :])
            else:
                t = g.tile(list(shape), f32, name=name)
                for sl, srcap in pieces:
                    nc.sync.dma_start(t[sl], srcap)
            return t

        SALL = (slice(None), slice(None))
        w_c1w1a = g.tile([3, 64], f32); nc.sync.dma_start(w_c1w1a[:], c1w1[0:3, :])
        w_c1w1b = g.tile([3, 64], f32); nc.sync.dma_start(w_c1w1b[:], c1w1[3:6, :])
        w_c2w1a = g.tile([64, 128], f32); nc.sync.dma_start(w_c2w1a[:], c2w1[0:64, :])
        w_c2w1b = g.tile([64, 128], f32); nc.sync.dma_start(w_c2w1b[:], c2w1[64:128, :])
        w_c1w2 = load_w("w_c1w2", [64, 64], [(SALL, c1w2[:, :])])
        w_c1w3 = load_w("w_c1w3", [64, 64], [(SALL, c1w3[:, :])])
        w_c2w2 = load_w("w_c2w2", [128, 128], [(SALL, c2w2[:, :])], wdt=bf16)
        w_c2w3 = load_w("w_c2w3", [128, 256], [(SALL, c2w3[:, :])], wdt=bf16)
        w_l0w = load_w("w_l0w", [128, 1024],
                       [((slice(None), slice(0, 512)), l0w[0:128, :]),
                        ((slice(None), slice(512, 1024)), l0w[128:256, :])])
        w_l1w = load_w("w_l1w", [128, 1024],
                       [((slice(None), slice(c * 256, (c + 1) * 256)),
                         l1w[c * 128:(c + 1) * 128, :]) for c in range(4)])
        w_l2w = load_w("w_l2w", [128, 512],
                       [((slice(None), slice(0, 256)), l2w[0:128, :]),
                        ((slice(None), slice(256, 512)), l2w[128:256, :])])
        w_l3w = load_w("w_l3w", [128, 2 * NUM_CLASSES],
                       [((slice(None), slice(0, NUM_CLASSES)), l3w[0:128, :]),
                        ((slice(None), slice(NUM_CLASSES, 2 * NUM_CLASSES)), l3w[128:256, :])])

        # first-layer combined weights: a = x @ (W1a - W1b) + b  (point-major)
        W1comb = g.tile([4, 64], f32, name="W1comb")
        nc.vector.tensor_sub(W1comb[0:3, :], w_c1w1a[:, :], w_c1w1b[:, :])
        nc.sync.dma_start(W1comb[3:4, :], c1b1.rearrange("(o c) -> o c", o=1))
        W2comb = g.tile([65, 128], f32, name="W2comb")
        nc.vector.tensor_sub(W2comb[0:64, :], w_c2w1a[:, :], w_c2w1b[:, :])
        nc.sync.dma_start(W2comb[64:65, :], c2b1.rearrange("(o c) -> o c", o=1))

        def col(name, src, n):
            t = g.tile([n, 1], f32, name=name)
            nc.sync.dma_start(t[:, :], src.rearrange("(c o) -> c o", o=1))
            return t

        b_c1b2 = col("b_c1b2", c1b2, 64)
        b_c1b3 = col("b_c1b3", c1b3, 64)
        b_c2b2 = col("b_c2b2", c2b2, 128)
        b_c2b3 = g.tile([128, 2], f32)
        nc.sync.dma_start(b_c2b3[:, 0:1], c2b3.rearrange("(h c o) -> h c o", h=2, o=1)[0])
        nc.sync.dma_start(b_c2b3[:, 1:2], c2b3.rearrange("(h c o) -> h c o", h=2, o=1)[1])
        b_l0b = g.tile([128, 4], f32)
        for t_ in range(4):
            nc.sync.dma_start(b_l0b[:, t_:t_ + 1],
                              l0b.rearrange("(h c o) -> h c o", h=4, o=1)[t_])
        b_l1b = g.tile([128, 2], f32)
        for t_ in range(2):
            nc.sync.dma_start(b_l1b[:, t_:t_ + 1],
                              l1b.rearrange("(h c o) -> h c o", h=2, o=1)[t_])
        b_l2b = g.tile([128, 2], f32)
        for t_ in range(2):
            nc.sync.dma_start(b_l2b[:, t_:t_ + 1],
                              l2b.rearrange("(h c o) -> h c o", h=2, o=1)[t_])
        b_l3b = col("b_l3b", l3b, NUM_CLASSES)

        wraw_ctx.close()

        # engines cannot address partition bases 3/64 directly: stage a ones
        # row at partition 0 and DMA it into place
        ones_row = g.tile([1, N], f32, name="ones_row")
        nc.vector.memset(ones_row[:, :], 1.0)
        nc.sync.dma_start(A1[3:4, :], ones_row[:, :])
        nc.sync.dma_start(A2[64:65, :], ones_row[:, :])
        # (re-rounded to f32r in conv2 prep, after conv1 has run)

        # =============== conv1 prep ===============
        with tc.tile_pool(name="prep", bufs=2) as pp, \
             tc.tile_pool(name="prep_ps", bufs=2, space="PSUM") as ppp:
            scol = g.tile([128, NT], f32, name="scol1_pos")
            for i in range(NT):
                isl = slice(i * P, (i + 1) * P)
                pt = pp.tile([128, 3], f32, name="pt")
                nc.sync.dma_start(pt[:], pos[isl, :])
                sq = pp.tile([128, 3], f32, name="sq")
                nc.scalar.activation(sq[:], pt[:], ACT.Square,
                                     accum_out=scol[:, i:i + 1])
                tp = ppp.tile([3, 128], f32, name="tp", space="PSUM", bufs=1)
                nc.tensor.transpose(tp[:], pt[:], ident_sb[:])
                nc.scalar.copy(mmso(A1[0:3, isl]), tp[:])
            nc.scalar.mul(nscol1[:, :], scol[:, :], -1.0)
            nc.scalar.mul(mmso(B1[0:3, :]), A1[0:3, :], -2.0)
            # s row: transpose scol [128, NT] -> [NT, 128], stage in SBUF, then
            # one cross-partition DMA into B1 row 3 (engines cannot shift
            # partitions; DMA can).
            stp = ppp.tile([NT, 128], f32, name="stp", space="PSUM", bufs=1)
            nc.tensor.transpose(stp[:], scol[:, :], ident_sb[:])
            srow_sb = pp.tile([NT, 128], f32, name="srow_sb")
            nc.scalar.copy(srow_sb[:, :], stp[:, :])
            nc.sync.dma_start(
                B1[3:4, :].rearrange("o (p n) -> o p n", p=NT), srow_sb[:, :])

            # a_i point-major: one small matmul per tile (A1 rows carry [x;1])
            for i in range(NT):
                isl = slice(i * P, (i + 1) * P)
                pa = ppp.tile([128, 64], f32, name="pa", space="PSUM")
                nc.tensor.matmul(pa[:], A1[0:4, isl], W1comb[:, :])
                nc.scalar.copy(apm1[:, i * 64:(i + 1) * 64], pa[:])

            # v1 = x @ W1b, channel-major; rows -> DRAM [N, 64]
            for c in range(4):
                cs = slice(c * 512, (c + 1) * 512)
                pv = ppp.tile([64, 512], f32, name="pv", space="PSUM")
                nc.tensor.matmul(pv[:], w_c1w1b[:, :], A1[0:3, cs])
                nc.scalar.copy(vscratch[0:64, cs], pv[:])
            for grp in range(4):
                vstage = pp.tile([128, 256], f32, name="vstage")
                for m in range(4):
                    i = grp * 4 + m
                    tvp = ppp.tile([128, 64], f32, name="tvp", space="PSUM")
                    nc.tensor.transpose(tvp[:], vscratch[0:64, i * P:(i + 1) * P],
                                        ident_sb[0:64, 0:64])
                    nc.vector.tensor_copy(vstage[:, m * 64:(m + 1) * 64], tvp[:])
                nc.sync.dma_start(
                    v1d[:, :].rearrange("(g m r) ch -> g r m ch", g=4, m=4)[grp],
                    vstage[:, :])

        # =============== edge-conv block (shared structure) ===============
        def edge_conv(conv, sp, spp, post_tile=None):
            """conv=1: H=64 channels; conv=2: H=128 (256 out)."""
            if conv == 1:
                H, CON, WD, ED, idt = 64, 4, 64, f32, ident_sb
                Asb, Bsb, vd, nscol, apm = A1, B1, v1d, nscol1, apm1
                wl2, wl3 = w_c1w2, w_c1w3
                bl2, bl3 = b_c1b2, b_c1b3
                W = mm        # f32r bitcast for conv1 MLP
            else:
                H, CON, WD, ED, idt = 128, 65, 128, bf16, ident_bf
                Asb, Bsb, vd, nscol, apm = A2, B2, v2d, nscol2, apm2
                wl2, wl3 = w_c2w2, w_c2w3
                bl2, bl3 = b_c2b2, b_c2b3
                W = lambda ap: ap   # tiles already bf16
            nhalf = 1 if conv == 1 else 2

            state = {}

            def stage_scores(i):
                isl = slice(i * P, (i + 1) * P)
                key = keybufs[i % KB]
                keyh = key.bitcast(f16).rearrange("p (n two) -> p n two", two=2)
                for c in range(4):
                    cs = slice(c * 512, (c + 1) * 512)
                    psc = spp.tile([128, 512], f32, name="psc", tag="psc", bufs=2)
                    nc.tensor.matmul(psc[:, :],
                                     mms(Asb[0:CON, isl]), mms(Bsb[0:CON, cs]))
                    # fp16(-d2) into the high u16 halves (strided store)
                    nc.scalar.activation(
                        keyh[:, cs, 1:2],
                        psc[:, :].rearrange("p (n o) -> p n o", o=1),
                        ACT.Identity, bias=nscol[:, i:i + 1], scale=-1.0)
                state[i] = {"key": key}

            def stage_topk(i):
                # 3 rounds of max8 over the composite keys; match_replace
                # writes into a scratch copy so the key iota survives.  The
                # neighbor index is the low 16 bits of each winning key.
                key32 = state[i]["key"].bitcast(f32)
                vals = sp.tile([128, KSEL], u32, name="vals", tag="vals", bufs=4)
                valsf = vals.bitcast(f32)
                idx = sp.tile([128, KSEL], u32, name="idx", tag="idx", bufs=4)
                scr = sp.tile([128, N], f32, name="scr", tag="scr", bufs=1)
                for r in range(3):
                    rs = slice(r * 8, (r + 1) * 8)
                    src = key32 if r == 0 else scr[:, :]
                    nc.vector.max(valsf[:, rs], src)
                    if r < 2:
                        nc.vector.match_replace(scr[:, :], valsf[:, rs], src,
                                                NEG_BIG)
                    nc.vector.tensor_scalar(idx[:, rs], vals[:, rs],
                                            0x7FF, None, op0=OP.bitwise_and)
                state[i].update(idx=idx)

            def stage_gather(i):
                # HW indirect DMA consumes ONE offset per destination
                # partition, so issue one gather per neighbor slot.  Runs a
                # full pipeline stage ahead of the consumer so the ~1.1us
                # per-call GPSIMD descriptor-gen cost overlaps tile i-1's
                # MLP instead of stalling it.
                idx = state[i]["idx"]
                gath = sp.tile([128, KNN * WD], ED, name="gath", tag="gath", bufs=3)
                # rank 0 of the composite keys is the self point (d2=0) up to
                # fp noise, and its v rows are the tile's own contiguous rows:
                # fetch slot 0 with one regular HWDGE DMA (no GPSIMD cost).
                nc.sync.dma_start(gath[:, 0:WD], vd[i * P:(i + 1) * P, :])
                for k in range(1, KNN):
                    nc.gpsimd.indirect_dma_start(
                        out=gath[:, k * WD:(k + 1) * WD], out_offset=None,
                        in_=vd[:, :],
                        in_offset=bass.IndirectOffsetOnAxis(
                            ap=idx[:, k:k + 1], axis=0),
                    )
                state[i].update(gath=gath)

            def stage_mlp_a(i):
                # z1 = relu(a_i + v_j) point-major, then PE transposes to
                # channel-major; L2; L3 matmuls for half 0.
                isl = slice(i * P, (i + 1) * P)
                gath = state[i]["gath"]
                gv = gath.rearrange("p (k c) -> p k c", k=KNN)
                av = apm[:, i * WD:(i + 1) * WD] \
                    .rearrange("p (o c) -> p o c", o=1) \
                    .to_broadcast([128, KNN, WD])
                nc.vector.tensor_add(gv, gv, av)
                nc.scalar.activation(gath[:, :], gath[:, :], ACT.Relu)

                z1T = sp.tile([H, KNN * 128], ED, name="z1T", tag="z1T", bufs=2)
                for c in range(5):
                    cs = slice(c * 512, (c + 1) * 512)
                    ptr = spp.tile([H, 512], ED, name="ptr", tag="ptr", bufs=1,
                                   space="PSUM")
                    for m in range(4):
                        k = c * 4 + m
                        nc.tensor.transpose(
                            ptr[:, m * 128:(m + 1) * 128],
                            gath[:, k * WD:(k + 1) * WD],
                            idt[:, :])
                    nc.scalar.copy(W(z1T[:, cs]), ptr[:, :])
                # ---- layer 2 ----
                z2T = sp.tile([H, KNN * 128], ED, name="z2T", tag="z2T", bufs=2)
                for c in range(5):
                    cs = slice(c * 512, (c + 1) * 512)
                    pm = spp.tile([H, 512], f32, name="pm", tag="pm", bufs=1,
                                  space="PSUM")
                    nc.tensor.matmul(pm[:], W(wl2[:, :]), W(z1T[:, cs]))
                    nc.scalar.activation(W(z2T[:, cs]), pm[:], ACT.Relu,
                                         bias=bl2[:, 0:1])
                # ---- layer 3, half 0: chunks 0-3 into one 4-bank PSUM
                # tile; chunk 4 borrows the pm-tag bank (L2 is done with it)
                wsel = wl3[:, :] if conv == 1 else wl3[:, 0:128]
                pl = spp.tile([H, 4 * 512], f32, name="pl", tag="pl", bufs=1,
                              space="PSUM")
                for c in range(4):
                    nc.tensor.matmul(pl[:, c * 512:(c + 1) * 512],
                                     W(wsel), W(z2T[:, c * 512:(c + 1) * 512]))
                pl4 = spp.tile([H, 512], f32, name="pl4", tag="pm", bufs=1,
                               space="PSUM")
                nc.tensor.matmul(pl4[:], W(wsel), W(z2T[:, 4 * 512:5 * 512]))
                state[i].update(z2T=z2T, pl=pl, pl4=pl4)

            def stage_mlp_b(i):
                # max over K for half 0 (one strided reduce over 5 banks),
                # then L3 matmuls for half 1 (conv2) reusing the same banks.
                isl = slice(i * P, (i + 1) * P)
                pl, pl4 = state[i]["pl"], state[i]["pl4"]
                redf = sp.tile([128, 128], f32, name="redf", tag="redf", bufs=2)
                red4 = sp.tile([128, 128], f32, name="red4", tag="red4", bufs=2)
                nc.vector.tensor_reduce(
                    redf[0:H, :],
                    pl[:, :].rearrange("p (c k n) -> p n (c k)", c=4, k=4),
                    axis=AX.X, op=OP.max)
                nc.vector.tensor_reduce(
                    red4[0:H, :],
                    pl4[:, :].rearrange("p (k n) -> p n k", k=4),
                    axis=AX.X, op=OP.max)
                nc.vector.tensor_tensor(redf[0:H, :], redf[0:H, :],
                                        red4[0:H, :], op=OP.max)
                if conv == 1:
                    nc.scalar.activation(mmso(A2[0:64, isl]), redf[0:64, :],
                                         ACT.Relu, bias=bl3[:, 0:1])
                else:
                    nc.scalar.activation(mmo(x2Ta[:, isl]), redf[:, :],
                                         ACT.Relu, bias=bl3[:, 0:1])
                    z2T = state[i]["z2T"]
                    pl2 = spp.tile([H, 4 * 512], f32, name="pl2", tag="pl",
                                   bufs=1, space="PSUM")
                    for c in range(4):
                        nc.tensor.matmul(
                            pl2[:, c * 512:(c + 1) * 512],
                            wl3[:, 128:256], W(z2T[:, c * 512:(c + 1) * 512]))
                    pl42 = spp.tile([H, 512], f32, name="pl42", tag="pm",
                                    bufs=1, space="PSUM")
                    nc.tensor.matmul(pl42[:], wl3[:, 128:256],
                                     W(z2T[:, 4 * 512:5 * 512]))
                    state[i]["pl2"] = pl2
                    state[i]["pl42"] = pl42

            def stage_mlp_c(i):
                if conv == 1:
                    del state[i]
                    return
                isl = slice(i * P, (i + 1) * P)
                pl2, pl42 = state[i]["pl2"], state[i]["pl42"]
                redf = sp.tile([128, 128], f32, name="redf", tag="redf", bufs=2)
                red4 = sp.tile([128, 128], f32, name="red4", tag="red4", bufs=2)
                nc.vector.tensor_reduce(
                    redf[:, :],
                    pl2[:, :].rearrange("p (c k n) -> p n (c k)", c=4, k=4),
                    axis=AX.X, op=OP.max)
                nc.vector.tensor_reduce(
                    red4[:, :],
                    pl42[:, :].rearrange("p (k n) -> p n k", k=4),
                    axis=AX.X, op=OP.max)
                nc.vector.tensor_tensor(redf[:, :], redf[:, :],
                                        red4[:, :], op=OP.max)
                nc.scalar.activation(mmo(x2Tb[:, isl]), redf[:, :],
                                     ACT.Relu, bias=bl3[:, 1:2])
                del state[i]

            # 3-deep software pipeline: while tile i's MLP runs, the GPSIMD
            # queue drains tile i+1's 20 gathers (issued a full stage after
            # their top-k) and the DVE runs tile i+2's top-k rounds, so no
            # engine waits on same-tile producers.
            stage_scores(0)
            stage_topk(0)
            stage_gather(0)
            if NT > 1:
                stage_scores(1)
                stage_topk(1)
            for i in range(NT):
                stage_mlp_a(i)
                if i + 2 < NT:
                    stage_scores(i + 2)
                if i + 1 < NT:
                    stage_gather(i + 1)
                stage_mlp_b(i)
                if i + 2 < NT:
                    stage_topk(i + 2)
                stage_mlp_c(i)
                if post_tile is not None:
                    post_tile(i)

        # =============== conv1 (+ conv2 prep interleaved) ===============
        # conv2's prep only needs A2 columns that conv1 has already written,
        # so emit it per 4-tile chunk inside conv1's pipeline: it fills the
        # PE/ACT idle slots and removes the serial ~100us transition phase.
        ones64 = g.tile([64, 1], f32, name="ones64")
        nc.vector.memset(ones64[:, :], 1.0)
        # reuse the setup-only ones_row tile's space for the s2 staging row
        # (its DMAs into A1/A2 happened long before conv1 writes land here)
        s2tmp = ones_row

        with tc.tile_pool(name="c1", bufs=2) as sp, \
             tc.tile_pool(name="c1ps", bufs=2, space="PSUM") as spp:

            def prep2_chunk(i):
                if i % 4 != 3:
                    return
                c = i // 4
                cs = slice(c * 512, (c + 1) * 512)
                # s2 for this chunk (square into vscratch, ones-matmul)
                nc.scalar.activation(vscratch[0:64, cs], A2[0:64, cs],
                                     ACT.Square)
                ps2 = spp.tile([1, 512], f32, name="ps2", tag="ptr", bufs=1,
                               space="PSUM")
                nc.tensor.matmul(ps2[:], ones64[:, :], vscratch[0:64, cs])
                nc.scalar.copy(s2tmp[0:1, cs], ps2[:])
                for m in range(4):
                    t = c * 4 + m
                    tsl = slice(t * P, (t + 1) * P)
                    tsc = spp.tile([128, 1], f32, name="tsc", tag="ptr",
                                   bufs=1, space="PSUM")
                    nc.tensor.transpose(tsc[:], s2tmp[0:1, tsl],
                                        ident_sb[0:1, 0:1])
                    nc.scalar.mul(nscol2[:, t:t + 1], tsc[:], -1.0)
                nc.scalar.mul(mmso(B2[0:64, cs]), A2[0:64, cs], -2.0)
                # a_i point-major for the 4 tiles of this chunk
                for m in range(4):
                    t = c * 4 + m
                    tsl = slice(t * P, (t + 1) * P)
                    pa2 = spp.tile([128, 128], f32, name="pa2", tag="ptr",
                                   bufs=1, space="PSUM")
                    nc.tensor.matmul(pa2[:], A2[0:65, tsl], W2comb[:, :])
                    nc.scalar.copy(apm2[:, t * 128:(t + 1) * 128], pa2[:])
                # v2 = x1 @ W2b for this chunk -> DRAM rows
                pv = spp.tile([128, 512], f32, name="pv2", tag="ptr", bufs=1,
                              space="PSUM")
                nc.tensor.matmul(pv[:], w_c2w1b[:, :], A2[0:64, cs])
                nc.scalar.copy(vscratch[:, cs], pv[:])
                vstage = sp.tile([128, 512], bf16, name="vstage2",
                                 tag="vstage2", bufs=2)
                for m in range(4):
                    t = c * 4 + m
                    tvp = spp.tile([128, 128], f32, name="tvp2", tag="ptr",
                                   bufs=1, space="PSUM")
                    nc.tensor.transpose(tvp[:], vscratch[:, t * P:(t + 1) * P],
                                        ident_sb[:, :])
                    nc.vector.tensor_copy(vstage[:, m * 128:(m + 1) * 128],
                                          tvp[:])
                nc.sync.dma_start(
                    v2d[:, :].rearrange("(g m r) ch -> g r m ch", g=4, m=4)[c],
                    vstage[:, :])

            edge_conv(1, sp, spp, post_tile=prep2_chunk)

        # B2 row 64 needs the full s2tmp (engines cannot address partition
        # base 64; DMA can)
        nc.sync.dma_start(B2[64:65, :], s2tmp[:, :])

        # =============== conv2 ===============
        with tc.tile_pool(name="c2", bufs=2) as sp, \
             tc.tile_pool(name="c2ps", bufs=2, space="PSUM") as spp:
            edge_conv(2, sp, spp)

        # =============== classifier ===============
        with tc.tile_pool(name="cls", bufs=2) as cp, \
             tc.tile_pool(name="clsps", bufs=2, space="PSUM") as cpp:
            pooled = g.tile([128, 4], f32, name="pooled")
            for t_ in range(4):
                tsl = slice(t_ * 128, (t_ + 1) * 128)
                ps = cpp.tile([128, 2048], f32, name="ps_l0", tag="ps_l0", bufs=1)
                for c in range(4):
                    cs = slice(c * 512, (c + 1) * 512)
                    nc.tensor.matmul(ps[:, cs], mm(w_l0w[:, 0:512][:, tsl]),
                                     mm(x2Ta[:, cs]), start=True, stop=False)
                    nc.tensor.matmul(ps[:, cs], mm(w_l0w[:, 512:1024][:, tsl]),
                                     mm(x2Tb[:, cs]), start=False, stop=True)
                pool1 = cp.tile([128, 1], f32, name="pool1")
                nc.vector.tensor_reduce(pool1[:, :], ps[:, :], axis=AX.X, op=OP.max)
                nc.scalar.activation(pooled[:, t_:t_ + 1], pool1[:, :],
                                     ACT.Relu, bias=b_l0b[:, t_:t_ + 1])
            # l1: 512 -> 256
            y1 = g.tile([128, 2], f32, name="y1")
            for h in range(2):
                ps1 = cpp.tile([128, 1], f32, name="ps_l1", tag="ps_s")
                for c in range(4):
                    nc.tensor.matmul(ps1[:],
                                     w_l1w[:, c * 256 + h * 128: c * 256 + (h + 1) * 128].bitcast(f32),
                                     pooled[:, c:c + 1],
                                     start=(c == 0), stop=(c == 3))
                nc.scalar.activation(y1[:, h:h + 1], ps1[:, :], ACT.Relu,
                                     bias=b_l1b[:, h:h + 1])
            # l2: 256 -> 256
            y2 = g.tile([128, 2], f32, name="y2")
            for h in range(2):
                ps2_ = cpp.tile([128, 1], f32, name="ps_l2", tag="ps_s")
                for c in range(2):
                    nc.tensor.matmul(ps2_[:],
                                     w_l2w[:, c * 256 + h * 128: c * 256 + (h + 1) * 128].bitcast(f32),
                                     y1[:, c:c + 1],
                                     start=(c == 0), stop=(c == 1))
                nc.scalar.activation(y2[:, h:h + 1], ps2_[:, :], ACT.Relu,
                                     bias=b_l2b[:, h:h + 1])
            # l3: 256 -> 40
            ps3 = cpp.tile([NUM_CLASSES, 1], f32, name="ps_l3", tag="ps_s")
            for c in range(2):
                nc.tensor.matmul(ps3[:],
                                 w_l3w[:, c * NUM_CLASSES:(c + 1) * NUM_CLASSES].bitcast(f32),
                                 y2[:, c:c + 1],
                                 start=(c == 0), stop=(c == 1))
            y3 = cp.tile([NUM_CLASSES, 1], f32, name="y3")
            nc.vector.tensor_add(y3[:, :], ps3[:, :], b_l3b[:, :])
            # log_softmax over the 40 values: transpose to one row
            pr = cpp.tile([1, NUM_CLASSES], f32, name="pr", tag="ps_s")
            nc.tensor.transpose(pr[:], y3[:, :], ident_sb[0:NUM_CLASSES, 0:NUM_CLASSES])
            row = cp.tile([1, NUM_CLASSES], f32, name="row")
            nc.vector.tensor_copy(row[:, :], pr[:, :])
            mx = cp.tile([1, 1], f32, name="mx")
            nc.vector.tensor_reduce(mx[:, :], row[:, :], axis=AX.X, op=OP.max)
            nmx = cp.tile([1, 1], f32, name="nmx")
            nc.scalar.mul(nmx[:, :], mx[:, :], -1.0)
            ex = cp.tile([1, NUM_CLASSES], f32, name="ex")
            sacc = cp.tile([1, 1], f32, name="sacc")
            nc.scalar.activation(ex[:, :], row[:, :], ACT.Exp,
                                 bias=nmx[:, 0:1], accum_out=sacc[:, :])
            lnz = cp.tile([1, 1], f32, name="lnz")
            nc.scalar.activation(lnz[:, :], sacc[:, :], ACT.Ln)
            shift = cp.tile([1, 1], f32, name="shift")
            nc.vector.tensor_sub(shift[:, :], lnz[:, :], nmx[:, :])
            osb = cp.tile([1, NUM_CLASSES], f32, name="osb")
            nc.vector.tensor_scalar(osb[:, :], row[:, :], shift[:, 0:1],
                                    None, op0=OP.subtract)
            nc.sync.dma_start(out[:, :], osb[:, :])

        ctx.close()

    nc.compile()
    return nc


def _get_program():
    if "nc" not in _PROGRAM_CACHE:
        _PROGRAM_CACHE["nc"] = _build_program()
    return _PROGRAM_CACHE["nc"]


def _in_maps(inputs):
    w_names = ["c1w1", "c1b1", "c1w2", "c1b2", "c1w3", "c1b3",
               "c2w1", "c2b1", "c2w2", "c2b2", "c2w3", "c2b3",
               "l0w", "l0b", "l1w", "l1b", "l2w", "l2b", "l3w", "l3b"]
    shared = {k: np.ascontiguousarray(np.asarray(inputs[k], np.float32))
              for k in w_names}
    shared["ident"] = np.eye(128, dtype=np.float32)
    pos = np.ascontiguousarray(np.asarray(inputs["pos"], np.float32))
    maps = []
    for c in range(NCLOUD):
        m = dict(shared)
        m["pos"] = np.ascontiguousarray(pos[c * N:(c + 1) * N])
        maps.append(m)
    return maps


def kernel(**inputs) -> np.ndarray:
    from concourse import bass_utils
    nc = _get_program()
    maps = _in_maps(inputs)
    res = bass_utils.run_bass_kernel_spmd(nc, maps, core_ids=list(range(NCLOUD)))
    outs = [np.asarray(r["out"]).reshape(1, NUM_CLASSES) for r in res.results]
    return np.concatenate(outs, axis=0).astype(np.float32)
